# revision 10
# baseline (speedup 1.0000x reference)
"""Trainium2 Bass kernel for nn_BatchTCLoss (beta-TCVAE ELBO loss).

Strategy (8 NeuronCores, data-parallel over the sample axis i):
  - Each core owns 64 of the 512 latent rows (and the matching 64 images for
    the BCE term); mu/logvar are replicated.
  - logqz_mat[i,j,k] = -0.5*((s_ik-mu_jk)^2*exp(lv_jk) + lv_jk + LOG2PI)
    expands as a_ik*w_jk + b_ik*g2_jk + c*q_jk with
      a = -0.5*s^2, b = s, c = -0.5
      w = exp(lv), g2 = mu*w, q = mu^2*w + lv + LOG2PI
    so each (i, k)-slice over all j is a rank-3 matmul.  Two k-slices are
    packed per 128x512 PSUM tile via a 6-row block-diagonal lhsT, giving
    full-width TensorE + ScalarE tiles.
  - Per-(i,k) logsumexp over j: exp on ScalarE (values are <= exp(-0.69), no
    max-subtraction needed), row-sum fused into VectorE tensor_scalar
    accumulators, log at the end.
  - logqz: S1[i,j] = sum_k logqz_mat via 5 accumulated matmuls, then a
    max-stabilized exp-sum on one 64x512 tile.
  - BCE + dimension-wise KL are streamed elementwise reductions.
  - Each core emits tiny per-core partial tensors; the host combines them
    (the final reduction is O(1000) flops).
"""

import numpy as np
from contextlib import ExitStack

import concourse.bass as bass
import concourse.tile as tile
from concourse import mybir
from concourse.masks import make_identity

B = 512          # batch
Z = 256          # latent dim
NCORES = 8
IB = B // NCORES   # 64 local samples per core
J = B              # pairwise j axis
P = 128            # partitions
KK = Z // 2        # 128 k-pairs (k, k+128)
CHW = 3 * 64 * 64
REC_F = IB * CHW // P   # 6144 free elems/partition of the image shard
RCH = 1024              # rec chunk (free elems per partition)
NRC = REC_F // RCH      # 6 chunks
LOG2PI = float(np.log(2.0 * np.pi))

f32 = mybir.dt.float32
bf16 = mybir.dt.bfloat16
AF = mybir.ActivationFunctionType
OP = mybir.AluOpType
AX = mybir.AxisListType




def _vmul(nc, out, a, b):
    # a*b via scalar_tensor_tensor: (a mult 1.0) mult b  (TT encoding has
    # only one sync-wait slot in walrus; TensorScalarPtr has more)
    nc.vector.scalar_tensor_tensor(out, a, 1.0, b, OP.mult, OP.mult)


def _vadd(nc, out, a, b):
    nc.vector.scalar_tensor_tensor(out, a, 0.0, b, OP.add, OP.add)


def _vcopy(nc, out, in_):
    nc.vector.tensor_scalar(out, in_, 0.0, None, OP.add)


def _split_multi_waits(nc):
    """This container's walrus accepts only ONE embedded sync-wait per
    compute/DMA instruction ("Too many sync wait commands").  Hoist extra
    waits onto same-engine NoOp carriers inserted immediately before the
    instruction — engines execute their stream in order, so this is
    semantics-preserving."""
    wid = 0
    for f in nc.m.functions:
        for blk in f.blocks:
            il = blk.instructions
            i = 0
            while i < len(il):
                ins = il[i]
                si = ins.sync_info
                tname = type(ins).__name__
                if si is not None and len(si.on_wait) > 1 and tname != "InstNoOp":
                    waits = list(si.on_wait)
                    nops = []
                    for w in waits[:-1]:
                        nop = mybir.InstNoOp(name=f"WSPLIT-{wid}", ins=[],
                                             outs=[], text_hint="wait_split")
                        wid += 1
                        nop.engine = ins.engine
                        nop.sync_info = mybir.SyncInfo(on_wait=[w], on_update=[])
                        nc.register_instruction(nop, overwrite=True)
                        nops.append(nop)
                    ins.sync_info = mybir.SyncInfo(on_wait=[waits[-1]],
                                                   on_update=list(si.on_update))
                    for j, nop in enumerate(nops):
                        il.insert(i + j, nop)
                    i += len(nops)
                i += 1
    return nc


def build_program():
    nc = bass.Bass("TRN2", target_bir_lowering=False, debug=False)

    d_mu = nc.dram_tensor("mu", [B, Z], f32, kind="ExternalInput").ap()
    d_lv = nc.dram_tensor("lv", [B, Z], f32, kind="ExternalInput").ap()
    d_lat = nc.dram_tensor("lat", [IB, Z], f32, kind="ExternalInput").ap()
    d_data = nc.dram_tensor("data", [P, REC_F], f32, kind="ExternalInput").ap()
    d_rec = nc.dram_tensor("recon", [P, REC_F], f32, kind="ExternalInput").ap()

    o_pm = nc.dram_tensor("o_pm", [P, 1], f32, kind="ExternalOutput").ap()
    o_s1 = nc.dram_tensor("o_s1", [IB, 2], f32, kind="ExternalOutput").ap()
    o_rec = nc.dram_tensor("o_rec", [P, NRC * 3], f32, kind="ExternalOutput").ap()
    o_dwkl = nc.dram_tensor("o_dwkl", [P, 2], f32, kind="ExternalOutput").ap()

    with tile.TileContext(nc) as tc, ExitStack() as ctx:
        keep = ctx.enter_context(tc.tile_pool(name="keep", bufs=1))

        identity = keep.tile([P, P], bf16)
        make_identity(nc, identity)
        ones_col = keep.tile([P, 1], bf16)
        nc.vector.memset(ones_col, 1.0)
        mhalf_row = keep.tile([1, IB], bf16)
        nc.vector.memset(mhalf_row, -0.5)

        # transposed (k-major) coefficient tensors; dim1 = k half (k, k+128)
        WT = keep.tile([P, 2, J], bf16)
        G2T = keep.tile([P, 2, J], bf16)
        QT = keep.tile([P, 2, J], bf16)
        AT = keep.tile([P, 2, IB], bf16)
        BT = keep.tile([P, 2, IB], bf16)

        LHS = keep.tile([6, KK * P], bf16)   # block-diagonal stationary tiles
        RHS = keep.tile([6, KK * J], bf16)   # interleaved moving tiles
        A_red = keep.tile([P, KK], f32)      # sum_j exp(logqz_mat), (i,khalf) x kk
        sink = keep.tile([P, J], bf16)       # dead main-out for accumulating ops
        ACCR = keep.tile([P, NRC * 3], f32)  # rec partial sums

        LHSv = LHS.rearrange("r (g n) -> r g n", g=KK)
        RHSv = RHS.rearrange("r (g n) -> r g n", g=KK)

        # ---------------- prep ----------------
        with tc.tile_pool(name="prep", bufs=1) as prep:
            MU = prep.tile([P, 4, Z], f32)
            nc.sync.dma_start(MU, d_mu.rearrange("(t p) k -> p t k", p=P))
            LVt = prep.tile([P, 4, Z], f32)
            nc.sync.dma_start(LVt, d_lv.rearrange("(t p) k -> p t k", p=P))
            S0 = prep.tile([IB, Z], f32)
            nc.sync.dma_start(S0, d_lat)
            MUf = MU.rearrange("p t k -> p (t k)")
            LVf = LVt.rearrange("p t k -> p (t k)")

            # dimension-wise KL partials: sum exp(mu^2+lv) and sum lv
            DW = prep.tile([P, 2], f32)
            MSQ = prep.tile([P, 4 * Z], f32)
            _vmul(nc, MSQ, MUf, MUf)
            _vadd(nc, MSQ, MSQ, LVf)
            nc.scalar.activation(MSQ, MSQ, AF.Exp, accum_out=DW[:, 0:1])
            nc.vector.tensor_scalar(MSQ, LVf, 1.0, None, OP.mult, OP.add,
                                    accum_out=DW[:, 1:2])
            nc.sync.dma_start(o_dwkl, DW)

            # per-(j,k) coefficients
            WS = prep.tile([P, 4 * Z], f32)
            nc.scalar.activation(WS, LVf, AF.Exp)
            G2S = prep.tile([P, 4 * Z], f32)
            _vmul(nc, G2S, MUf, WS)
            QS = prep.tile([P, 4 * Z], f32)
            _vmul(nc, QS, MUf, G2S)
            nc.vector.scalar_tensor_tensor(QS, QS, LOG2PI, LVf, OP.add, OP.add)
            wb = prep.tile([P, 4, Z], bf16)
            _vcopy(nc, wb.rearrange("p t k -> p (t k)"), WS)
            g2b = prep.tile([P, 4, Z], bf16)
            _vcopy(nc, g2b.rearrange("p t k -> p (t k)"), G2S)
            qb = prep.tile([P, 4, Z], bf16)
            _vcopy(nc, qb.rearrange("p t k -> p (t k)"), QS)

            # per-(i,k) coefficients
            SSQ = prep.tile([IB, Z], f32)
            _vmul(nc, SSQ, S0, S0)
            ab = prep.tile([IB, Z], bf16)
            nc.vector.tensor_scalar_mul(ab, SSQ, -0.5)
            sb = prep.tile([IB, Z], bf16)
            _vcopy(nc, sb, S0)

            # transpose to k-major via TensorE
            with tc.tile_pool(name="tpsum", bufs=3, space="PSUM") as tpsum:
                for srcb, dstT in ((wb, WT), (g2b, G2T), (qb, QT)):
                    for kh in range(2):
                        for jt in range(4):
                            pt = tpsum.tile([P, P], bf16, tag="tp")
                            nc.tensor.transpose(
                                pt, srcb[:, jt, kh * 128:(kh + 1) * 128], identity)
                            _vcopy(nc, dstT[:, kh, jt * 128:(jt + 1) * 128], pt)
                for srcb, dstT in ((ab, AT), (sb, BT)):
                    for kh in range(2):
                        pt = tpsum.tile([P, P], bf16, tag="tp")
                        nc.tensor.transpose(
                            pt[:, 0:IB], srcb[:, kh * 128:(kh + 1) * 128],
                            identity[0:IB, 0:IB])
                        _vcopy(nc, dstT[:, kh, :], pt[:, 0:IB])

            # gather into the interleaved moving/stationary buffers
            for r, (src, kh) in enumerate(
                    ((WT, 0), (G2T, 0), (QT, 0), (WT, 1), (G2T, 1), (QT, 1))):
                nc.sync.dma_start(RHSv[r:r + 1], src[:, kh, :])
            nc.vector.memset(LHS, 0.0)
            nc.sync.dma_start(LHSv[0:1, :, 0:IB], AT[:, 0, :])
            nc.sync.dma_start(LHSv[1:2, :, 0:IB], BT[:, 0, :])
            nc.sync.dma_start(LHSv[3:4, :, IB:P], AT[:, 1, :])
            nc.sync.dma_start(LHSv[4:5, :, IB:P], BT[:, 1, :])
            # const -0.5 coefficient rows (rows 2 and 5 have base partitions
            # an engine op can't target, so fill them with broadcast DMAs)
            mhalf_bcast = bass.AP(tensor=mhalf_row.tensor, offset=mhalf_row.offset,
                                  ap=[list(mhalf_row.ap[0]), [0, KK], [1, IB]])
            nc.sync.dma_start(LHSv[2:3, :, 0:IB], mhalf_bcast)
            nc.sync.dma_start(LHSv[5:6, :, IB:P], mhalf_bcast)

            # ---------------- logqz path (S1 = sum_k logqz_mat) ----------------
            with tc.tile_pool(name="s1psum", bufs=1, space="PSUM") as s1p:
                qpv = s1p.tile([1, J], f32)
                nc.tensor.matmul(qpv, ones_col, QT[:, 0, :], start=True, stop=False)
                nc.tensor.matmul(qpv, ones_col, QT[:, 1, :], start=False, stop=True)
                qvS = prep.tile([1, J], bf16)
                _vcopy(nc, qvS, qpv)

                S1 = s1p.tile([IB, J], f32)
                nc.tensor.matmul(S1, AT[:, 0, :], WT[:, 0, :], start=True, stop=False)
                nc.tensor.matmul(S1, BT[:, 0, :], G2T[:, 0, :], start=False, stop=False)
                nc.tensor.matmul(S1, AT[:, 1, :], WT[:, 1, :], start=False, stop=False)
                nc.tensor.matmul(S1, BT[:, 1, :], G2T[:, 1, :], start=False, stop=False)
                nc.tensor.matmul(S1, mhalf_row, qvS, start=False, stop=True)

                negmax = prep.tile([IB, 1], f32)
                nc.vector.tensor_reduce(negmax, S1, axis=AX.X, op=OP.max, negate=True)
                es = prep.tile([IB, J], bf16)
                OS1 = prep.tile([IB, 2], f32)
                nc.scalar.activation(es, S1, AF.Exp, bias=negmax, scale=1.0,
                                     accum_out=OS1[:, 1:2])
                _vcopy(nc, OS1[:, 0:1], negmax)
                nc.sync.dma_start(o_s1, OS1)

        # ---------------- main pairwise loop ----------------
        with tc.tile_pool(name="mpsum", bufs=2, space="PSUM") as mp, \
                tc.tile_pool(name="epool", bufs=2) as ep:
            for g in range(KK // 4):
                T4 = mp.tile([P, 4, J], f32)
                for c in range(4):
                    m = 4 * g + c
                    nc.tensor.matmul(T4[:, c, :], LHSv[:, m, :], RHSv[:, m, :],
                                     start=True, stop=True)
                E4 = ep.tile([P, 4, J], bf16)
                nc.scalar.activation(E4.rearrange("p c j -> p (c j)"),
                                     T4.rearrange("p c j -> p (c j)"), AF.Exp)
                for c in range(4):
                    m = 4 * g + c
                    nc.vector.tensor_scalar(sink, E4[:, c, :], 1.0, None,
                                            OP.mult, OP.add,
                                            accum_out=A_red[:, m:m + 1])

        LG = keep.tile([P, KK], f32)
        nc.scalar.activation(LG, A_red, AF.Ln)
        PM = keep.tile([P, 1], f32)
        nc.vector.reduce_sum(PM, LG, axis=AX.X)
        nc.sync.dma_start(o_pm, PM)

        # ---------------- reconstruction BCE ----------------
        with tc.tile_pool(name="rpool", bufs=2) as rp:
            for ch in range(NRC):
                sl = slice(ch * RCH, (ch + 1) * RCH)
                DD = rp.tile([P, RCH], f32)
                nc.sync.dma_start(DD, d_data[:, sl])
                RR = rp.tile([P, RCH], f32)
                nc.sync.dma_start(RR, d_rec[:, sl])
                LR = rp.tile([P, RCH], f32)
                nc.scalar.activation(LR, RR, AF.Ln)
                L1R = rp.tile([P, RCH], f32)
                nc.scalar.activation(L1R, RR, AF.Ln, bias=1.0, scale=-1.0,
                                     accum_out=ACCR[:, 3 * ch + 1:3 * ch + 2])
                # RR is dead after the two logs; reuse it as the dead main-out
                nc.vector.scalar_tensor_tensor(
                    RR, DD, 1.0, LR, OP.mult, OP.mult,
                    accum_out=ACCR[:, 3 * ch:3 * ch + 1])
                nc.vector.scalar_tensor_tensor(
                    RR, DD, -1.0, L1R, OP.mult, OP.mult,
                    accum_out=ACCR[:, 3 * ch + 2:3 * ch + 3])
        nc.sync.dma_start(o_rec, ACCR)

    return _split_multi_waits(nc)


def make_in_maps(data, recon, lat, mu, lv):
    in_maps = []
    for c in range(NCORES):
        sl = slice(c * IB, (c + 1) * IB)
        in_maps.append({
            "mu": np.ascontiguousarray(mu, np.float32),
            "lv": np.ascontiguousarray(lv, np.float32),
            "lat": np.ascontiguousarray(lat[sl], np.float32),
            "data": np.ascontiguousarray(
                np.asarray(data[sl], np.float32).reshape(P, REC_F)),
            "recon": np.ascontiguousarray(
                np.asarray(recon[sl], np.float32).reshape(P, REC_F)),
        })
    return in_maps


def combine(results, dataset_size):
    """results: list of 8 dicts with per-core output tensors."""
    log_norm = float(np.log(np.float32(B)) + np.log(np.float32(float(dataset_size))))

    rec_sum = sum(r["o_rec"].astype(np.float64).sum() for r in results)
    rec_loss = -rec_sum / B

    dw = results[0]["o_dwkl"].astype(np.float64)
    dwkl = (0.5 * dw[:, 0].sum() - 0.5 * dw[:, 1].sum() - 0.5 * B * Z) / B

    tc_total = 0.0
    for r in results:
        pmh = r["o_pm"].astype(np.float64).ravel()
        pm = pmh[:IB] + pmh[IB:]
        prodmarg = pm - Z * log_norm
        s1 = r["o_s1"].astype(np.float64)
        lq = (-s1[:, 0]) + np.log(s1[:, 1]) - log_norm
        tc_total += (lq - prodmarg).sum()
    tc_loss = tc_total / B

    return np.array(rec_loss + tc_loss + dwkl, dtype=np.float32)


def run_on_hw(inputs, trace=False):
    from concourse.bass_utils import run_bass_kernel_spmd

    nc = build_program()
    in_maps = make_in_maps(inputs["data"], inputs["recon_batch"],
                           inputs["latent_sample"], inputs["mu"],
                           inputs["logvar"])
    br = run_bass_kernel_spmd(nc, in_maps, list(range(NCORES)), trace=trace)
    elbo = combine(br.results, inputs["dataset_size"])
    return elbo, br


def kernel(**inputs):
    elbo, _ = run_on_hw(inputs, trace=False)
    return elbo


# revision 13
# speedup vs baseline: 1.2615x; 1.2615x over previous
"""Trainium2 Bass kernel for nn_BatchTCLoss (beta-TCVAE ELBO loss).

Strategy (8 NeuronCores, data-parallel over the sample axis i):
  - Each core owns 64 of the 512 latent rows (and the matching 64 images for
    the BCE term); mu/logvar are replicated.
  - logqz_mat[i,j,k] = -0.5*((s_ik-mu_jk)^2*exp(lv_jk) + lv_jk + LOG2PI)
    expands as a_ik*w_jk + b_ik*g2_jk + c*q_jk with
      a = -0.5*s^2, b = s, c = -0.5
      w = exp(lv), g2 = mu*w, q = mu^2*w + lv + LOG2PI
    so each (i, k)-slice over all j is a rank-3 matmul.  Two k-slices are
    packed per 128x512 PSUM tile via a 6-row block-diagonal lhsT, giving
    full-width TensorE + ScalarE tiles.
  - Per-(i,k) logsumexp over j: exp on ScalarE (values are <= exp(-0.69), no
    max-subtraction needed), row-sum fused into VectorE tensor_scalar
    accumulators, log at the end.
  - logqz: S1[i,j] = sum_k logqz_mat via 5 accumulated matmuls, then a
    max-stabilized exp-sum on one 64x512 tile.
  - BCE + dimension-wise KL are streamed elementwise reductions.
  - Each core emits tiny per-core partial tensors; the host combines them
    (the final reduction is O(1000) flops).
"""

import numpy as np
from contextlib import ExitStack

import concourse.bass as bass
import concourse.tile as tile
from concourse import mybir
from concourse.masks import make_identity

B = 512          # batch
Z = 256          # latent dim
NCORES = 8
IB = B // NCORES   # 64 local samples per core
J = B              # pairwise j axis
P = 128            # partitions
KK = Z // 2        # 128 k-pairs (k, k+128)
CHW = 3 * 64 * 64
REC_F = IB * CHW // P   # 6144 free elems/partition of the image shard
RCH = 1024              # rec chunk (free elems per partition)
NRC = REC_F // RCH      # 6 chunks
LOG2PI = float(np.log(2.0 * np.pi))

f32 = mybir.dt.float32
bf16 = mybir.dt.bfloat16
AF = mybir.ActivationFunctionType
OP = mybir.AluOpType
AX = mybir.AxisListType




def _vmul(nc, out, a, b):
    # a*b via scalar_tensor_tensor: (a mult 1.0) mult b  (TT encoding has
    # only one sync-wait slot in walrus; TensorScalarPtr has more)
    nc.vector.scalar_tensor_tensor(out, a, 1.0, b, OP.mult, OP.mult)


def _vadd(nc, out, a, b):
    nc.vector.scalar_tensor_tensor(out, a, 0.0, b, OP.add, OP.add)


def _vcopy(nc, out, in_):
    nc.vector.tensor_scalar(out, in_, 0.0, None, OP.add)


def _split_multi_waits(nc):
    """This container's walrus accepts only ONE embedded sync-wait per
    compute/DMA instruction ("Too many sync wait commands").  Hoist extra
    waits onto same-engine NoOp carriers inserted immediately before the
    instruction — engines execute their stream in order, so this is
    semantics-preserving."""
    wid = 0
    for f in nc.m.functions:
        for blk in f.blocks:
            il = blk.instructions
            i = 0
            while i < len(il):
                ins = il[i]
                si = ins.sync_info
                tname = type(ins).__name__
                if si is not None and len(si.on_wait) > 1 and tname != "InstNoOp":
                    waits = list(si.on_wait)
                    nops = []
                    for w in waits[:-1]:
                        nop = mybir.InstNoOp(name=f"WSPLIT-{wid}", ins=[],
                                             outs=[], text_hint="wait_split")
                        wid += 1
                        nop.engine = ins.engine
                        nop.sync_info = mybir.SyncInfo(on_wait=[w], on_update=[])
                        nc.register_instruction(nop, overwrite=True)
                        nops.append(nop)
                    ins.sync_info = mybir.SyncInfo(on_wait=[waits[-1]],
                                                   on_update=list(si.on_update))
                    for j, nop in enumerate(nops):
                        il.insert(i + j, nop)
                    i += len(nops)
                i += 1
    return nc


def build_program():
    nc = bass.Bass("TRN2", target_bir_lowering=False, debug=False)

    d_mu = nc.dram_tensor("mu", [B, Z], f32, kind="ExternalInput").ap()
    d_lv = nc.dram_tensor("lv", [B, Z], f32, kind="ExternalInput").ap()
    d_lat = nc.dram_tensor("lat", [IB, Z], f32, kind="ExternalInput").ap()
    d_data = nc.dram_tensor("data", [P, REC_F], f32, kind="ExternalInput").ap()
    d_rec = nc.dram_tensor("recon", [P, REC_F], f32, kind="ExternalInput").ap()

    o_pm = nc.dram_tensor("o_pm", [P, 1], f32, kind="ExternalOutput").ap()
    o_s1 = nc.dram_tensor("o_s1", [IB, 2], f32, kind="ExternalOutput").ap()
    o_rec = nc.dram_tensor("o_rec", [P, NRC * 3], f32, kind="ExternalOutput").ap()
    o_dwkl = nc.dram_tensor("o_dwkl", [P, 2], f32, kind="ExternalOutput").ap()

    HK = KK // 2  # 64 process indices per row-group half

    with tile.TileContext(nc) as tc, ExitStack() as ctx:
        keep = ctx.enter_context(tc.tile_pool(name="keep", bufs=1))

        identity = keep.tile([P, P], bf16)
        make_identity(nc, identity)
        ones_col = keep.tile([P, 1], bf16)
        nc.gpsimd.memset(ones_col, 1.0)
        mhalf_row = keep.tile([1, IB], bf16)
        nc.gpsimd.memset(mhalf_row, -0.5)

        # transposed (k-major) coefficient tensors; dim1 = k half (k, k+128)
        WT = keep.tile([P, 2, J], bf16)
        G2T = keep.tile([P, 2, J], bf16)
        QT = keep.tile([P, 2, J], bf16)
        AT = keep.tile([P, 2, IB], bf16)
        BT = keep.tile([P, 2, IB], bf16)

        # stationary (block-diag) + moving tiles, split into two partition
        # row-groups (base 0 / base 32) so LDWEIGHTS of one group overlaps
        # the in-flight matmul of the other (different PE row_grps).
        LHS_E = keep.tile([6, HK * P], bf16)
        RHS_E = keep.tile([6, HK * J], bf16)
        LHS_Of = keep.tile([38, HK * P], bf16)
        RHS_Of = keep.tile([38, HK * J], bf16)
        LHS_O = LHS_Of[32:38]
        RHS_O = RHS_Of[32:38]

        A_red = keep.tile([P, KK], f32)     # sum_j exp(logqz_mat)
        ACCR = keep.tile([P, NRC * 3], f32)  # rec partial sums
        qvS = keep.tile([1, J], bf16)
        OS1 = keep.tile([IB, 2], f32)
        negmax = keep.tile([IB, 1], f32)

        LHSvE = LHS_E.rearrange("r (g n) -> r g n", g=HK)
        RHSvE = RHS_E.rearrange("r (g n) -> r g n", g=HK)
        LHSvO = LHS_O.rearrange("r (g n) -> r g n", g=HK)
        RHSvO = RHS_O.rearrange("r (g n) -> r g n", g=HK)

        # zero-init the stationary buffers early (gpsimd; DVE memset is slow)
        nc.gpsimd.memset(LHS_E, 0.0)
        nc.gpsimd.memset(LHS_Of[32:38], 0.0)

        # ---------------- prep ----------------
        with tc.tile_pool(name="prep", bufs=1) as prep:
            MU = prep.tile([P, 4, Z], f32)
            nc.sync.dma_start(MU, d_mu.rearrange("(t p) k -> p t k", p=P))
            LVt = prep.tile([P, 4, Z], f32)
            nc.sync.dma_start(LVt, d_lv.rearrange("(t p) k -> p t k", p=P))
            S0 = prep.tile([IB, Z], f32)
            nc.sync.dma_start(S0, d_lat)
            MUf = MU.rearrange("p t k -> p (t k)")
            LVf = LVt.rearrange("p t k -> p (t k)")

            # per-(j,k) coefficients
            WS = prep.tile([P, 4 * Z], f32)
            nc.scalar.activation(WS, LVf, AF.Exp)
            G2S = prep.tile([P, 4 * Z], f32)
            _vmul(nc, G2S, MUf, WS)
            QS = prep.tile([P, 4 * Z], f32)
            _vmul(nc, QS, MUf, G2S)
            nc.vector.scalar_tensor_tensor(QS, QS, LOG2PI, LVf, OP.add, OP.add)
            wb = prep.tile([P, 4, Z], bf16)
            _vcopy(nc, wb.rearrange("p t k -> p (t k)"), WS)
            g2b = prep.tile([P, 4, Z], bf16)
            _vcopy(nc, g2b.rearrange("p t k -> p (t k)"), G2S)
            qb = prep.tile([P, 4, Z], bf16)
            _vcopy(nc, qb.rearrange("p t k -> p (t k)"), QS)

            # per-(i,k) coefficients
            SSQ = prep.tile([IB, Z], f32)
            _vmul(nc, SSQ, S0, S0)
            ab = prep.tile([IB, Z], bf16)
            nc.vector.tensor_scalar_mul(ab, SSQ, -0.5)
            sb = prep.tile([IB, Z], bf16)
            _vcopy(nc, sb, S0)

            # dimension-wise KL partials: sum exp(mu^2+lv) and sum lv
            DW = prep.tile([P, 2], f32)
            MSQ = prep.tile([P, 4 * Z], f32)
            _vmul(nc, MSQ, MUf, MUf)
            _vadd(nc, MSQ, MSQ, LVf)
            nc.scalar.activation(MSQ, MSQ, AF.Exp, accum_out=DW[:, 0:1])
            nc.vector.tensor_scalar(MSQ, LVf, 1.0, None, OP.mult, OP.add,
                                    accum_out=DW[:, 1:2])
            nc.sync.dma_start(o_dwkl, DW)

            # transpose to k-major via TensorE
            with tc.tile_pool(name="tpsum", bufs=3, space="PSUM") as tpsum:
                for srcb, dstT in ((wb, WT), (g2b, G2T), (qb, QT)):
                    for kh in range(2):
                        for jt in range(4):
                            pt = tpsum.tile([P, P], bf16, tag="tp")
                            nc.tensor.transpose(
                                pt, srcb[:, jt, kh * 128:(kh + 1) * 128], identity)
                            _vcopy(nc, dstT[:, kh, jt * 128:(jt + 1) * 128], pt)
                for srcb, dstT in ((ab, AT), (sb, BT)):
                    for kh in range(2):
                        pt = tpsum.tile([P, P], bf16, tag="tp")
                        nc.tensor.transpose(
                            pt[:, 0:IB], srcb[:, kh * 128:(kh + 1) * 128],
                            identity[0:IB, 0:IB])
                        _vcopy(nc, dstT[:, kh, :], pt[:, 0:IB])

            # gather into interleaved buffers; spread across DMA queues.
            # process index m: even -> E group (WT partition m//2),
            #                  odd  -> O group (WT partition 64 + m//2)
            dq = [nc.sync, nc.scalar, nc.gpsimd]
            qi = 0
            for dst, psl in ((RHSvE, slice(0, HK)), (RHSvO, slice(HK, P))):
                for r, (src, kh) in enumerate(
                        ((WT, 0), (G2T, 0), (QT, 0), (WT, 1), (G2T, 1), (QT, 1))):
                    dq[qi % len(dq)].dma_start(dst[r:r + 1], src[psl, kh, :])
                    qi += 1
            for dst, psl in ((LHSvE, slice(0, HK)), (LHSvO, slice(HK, P))):
                dq[qi % len(dq)].dma_start(dst[0:1, :, 0:IB], AT[psl, 0, :]); qi += 1
                dq[qi % len(dq)].dma_start(dst[1:2, :, 0:IB], BT[psl, 0, :]); qi += 1
                dq[qi % len(dq)].dma_start(dst[3:4, :, IB:P], AT[psl, 1, :]); qi += 1
                dq[qi % len(dq)].dma_start(dst[4:5, :, IB:P], BT[psl, 1, :]); qi += 1
            mhalf_bcast = bass.AP(tensor=mhalf_row.tensor, offset=mhalf_row.offset,
                                  ap=[list(mhalf_row.ap[0]), [0, HK], [1, IB]])
            for dst in (LHSvE, LHSvO):
                dq[qi % len(dq)].dma_start(dst[2:3, :, 0:IB], mhalf_bcast); qi += 1
                dq[qi % len(dq)].dma_start(dst[5:6, :, IB:P], mhalf_bcast); qi += 1

        # ---------------- main pairwise loop (rec BCE interleaved) --------
        NG = KK // 4
        rec_at = {2 + 5 * c: c for c in range(NRC)}  # group idx -> rec chunk
        with tc.tile_pool(name="mpsum", bufs=2, space="PSUM") as mp, \
                tc.tile_pool(name="epool", bufs=2) as ep, \
                tc.tile_pool(name="rpool", bufs=2) as rp, \
                tc.tile_pool(name="rpool1", bufs=1) as rp1:
            for g in range(NG):
                T4 = mp.tile([P, 4, J], f32)
                for c in range(4):
                    m = 4 * g + c
                    h = m // 2
                    if m % 2 == 0:
                        lhs, rhs = LHSvE[:, h, :], RHSvE[:, h, :]
                    else:
                        lhs, rhs = LHSvO[:, h, :], RHSvO[:, h, :]
                    nc.tensor.matmul(T4[:, c, :], lhs, rhs, start=True, stop=True)
                E4 = ep.tile([P, 4, J], bf16)
                nc.scalar.activation(E4.rearrange("p c j -> p (c j)"),
                                     T4.rearrange("p c j -> p (c j)"), AF.Exp)
                # halving-tree sum over j (bf16 2x tensor-tensor adds), then a
                # single fp32 reduce of the 8-wide stumps into A_red columns
                hh = J // 2
                while hh >= 8:
                    nc.vector.tensor_add(E4[:, :, 0:hh], E4[:, :, 0:hh],
                                         E4[:, :, hh:2 * hh])
                    hh //= 2
                nc.vector.tensor_reduce(A_red[:, 4 * g:4 * g + 4], E4[:, :, 0:8],
                                        axis=AX.X, op=OP.add)

                if g in rec_at:
                    ch = rec_at[g]
                    sl = slice(ch * RCH, (ch + 1) * RCH)
                    DD = rp.tile([P, RCH], f32)
                    nc.gpsimd.dma_start(DD, d_data[:, sl])
                    RR = rp.tile([P, RCH], f32)
                    nc.gpsimd.dma_start(RR, d_rec[:, sl])
                    LR = rp1.tile([P, RCH], f32)
                    nc.scalar.activation(LR, RR, AF.Ln)
                    L1R = rp1.tile([P, RCH], f32)
                    nc.scalar.activation(L1R, RR, AF.Ln, bias=1.0, scale=-1.0,
                                         accum_out=ACCR[:, 3 * ch + 1:3 * ch + 2])
                    nc.vector.scalar_tensor_tensor(
                        RR, DD, 1.0, LR, OP.mult, OP.mult,
                        accum_out=ACCR[:, 3 * ch:3 * ch + 1])
                    nc.vector.scalar_tensor_tensor(
                        RR, DD, -1.0, L1R, OP.mult, OP.mult,
                        accum_out=ACCR[:, 3 * ch + 2:3 * ch + 3])
        nc.sync.dma_start(o_rec, ACCR)

        LG = keep.tile([P, KK], f32)
        nc.scalar.activation(LG, A_red, AF.Ln)
        PM = keep.tile([P, 1], f32)
        nc.vector.reduce_sum(PM, LG, axis=AX.X)
        nc.sync.dma_start(o_pm, PM)

        # ---------------- logqz path (S1 = sum_k logqz_mat) ----------------
        with tc.tile_pool(name="s1psum", bufs=1, space="PSUM") as s1p, \
                tc.tile_pool(name="s1sb", bufs=1) as s1sb:
            qpv = s1p.tile([1, J], f32)
            nc.tensor.matmul(qpv, ones_col, QT[:, 0, :], start=True, stop=False)
            nc.tensor.matmul(qpv, ones_col, QT[:, 1, :], start=False, stop=True)
            _vcopy(nc, qvS, qpv)

            S1 = s1p.tile([IB, J], f32)
            nc.tensor.matmul(S1, AT[:, 0, :], WT[:, 0, :], start=True, stop=False)
            nc.tensor.matmul(S1, BT[:, 0, :], G2T[:, 0, :], start=False, stop=False)
            nc.tensor.matmul(S1, AT[:, 1, :], WT[:, 1, :], start=False, stop=False)
            nc.tensor.matmul(S1, BT[:, 1, :], G2T[:, 1, :], start=False, stop=False)
            nc.tensor.matmul(S1, mhalf_row, qvS, start=False, stop=True)

            nc.vector.tensor_reduce(negmax, S1, axis=AX.X, op=OP.max, negate=True)
            es = s1sb.tile([IB, J], bf16)
            nc.scalar.activation(es, S1, AF.Exp, bias=negmax, scale=1.0,
                                 accum_out=OS1[:, 1:2])
            _vcopy(nc, OS1[:, 0:1], negmax)
            nc.sync.dma_start(o_s1, OS1)

    return _split_multi_waits(nc)


def make_in_maps(data, recon, lat, mu, lv):
    in_maps = []
    for c in range(NCORES):
        sl = slice(c * IB, (c + 1) * IB)
        in_maps.append({
            "mu": np.ascontiguousarray(mu, np.float32),
            "lv": np.ascontiguousarray(lv, np.float32),
            "lat": np.ascontiguousarray(lat[sl], np.float32),
            "data": np.ascontiguousarray(
                np.asarray(data[sl], np.float32).reshape(P, REC_F)),
            "recon": np.ascontiguousarray(
                np.asarray(recon[sl], np.float32).reshape(P, REC_F)),
        })
    return in_maps


def combine(results, dataset_size):
    """results: list of 8 dicts with per-core output tensors."""
    log_norm = float(np.log(np.float32(B)) + np.log(np.float32(float(dataset_size))))

    rec_sum = sum(r["o_rec"].astype(np.float64).sum() for r in results)
    rec_loss = -rec_sum / B

    dw = results[0]["o_dwkl"].astype(np.float64)
    dwkl = (0.5 * dw[:, 0].sum() - 0.5 * dw[:, 1].sum() - 0.5 * B * Z) / B

    tc_total = 0.0
    for r in results:
        pmh = r["o_pm"].astype(np.float64).ravel()
        pm = pmh[:IB] + pmh[IB:]
        prodmarg = pm - Z * log_norm
        s1 = r["o_s1"].astype(np.float64)
        lq = (-s1[:, 0]) + np.log(s1[:, 1]) - log_norm
        tc_total += (lq - prodmarg).sum()
    tc_loss = tc_total / B

    return np.array(rec_loss + tc_loss + dwkl, dtype=np.float32)


def run_on_hw(inputs, trace=False):
    from concourse.bass_utils import run_bass_kernel_spmd

    nc = build_program()
    in_maps = make_in_maps(inputs["data"], inputs["recon_batch"],
                           inputs["latent_sample"], inputs["mu"],
                           inputs["logvar"])
    br = run_bass_kernel_spmd(nc, in_maps, list(range(NCORES)), trace=trace)
    elbo = combine(br.results, inputs["dataset_size"])
    return elbo, br


def kernel(**inputs):
    elbo, _ = run_on_hw(inputs, trace=False)
    return elbo


# revision 14
# speedup vs baseline: 1.2669x; 1.0042x over previous
"""Trainium2 Bass kernel for nn_BatchTCLoss (beta-TCVAE ELBO loss).

Strategy (8 NeuronCores, data-parallel over the sample axis i):
  - Each core owns 64 of the 512 latent rows (and the matching 64 images for
    the BCE term); mu/logvar are replicated.
  - logqz_mat[i,j,k] = -0.5*((s_ik-mu_jk)^2*exp(lv_jk) + lv_jk + LOG2PI)
    expands as a_ik*w_jk + b_ik*g2_jk + c*q_jk with
      a = -0.5*s^2, b = s, c = -0.5
      w = exp(lv), g2 = mu*w, q = mu^2*w + lv + LOG2PI
    so each (i, k)-slice over all j is a rank-3 matmul.  Two k-slices are
    packed per 128x512 PSUM tile via a 6-row block-diagonal lhsT, giving
    full-width TensorE + ScalarE tiles.
  - Per-(i,k) logsumexp over j: exp on ScalarE (values are <= exp(-0.69), no
    max-subtraction needed), row-sum fused into VectorE tensor_scalar
    accumulators, log at the end.
  - logqz: S1[i,j] = sum_k logqz_mat via 5 accumulated matmuls, then a
    max-stabilized exp-sum on one 64x512 tile.
  - BCE + dimension-wise KL are streamed elementwise reductions.
  - Each core emits tiny per-core partial tensors; the host combines them
    (the final reduction is O(1000) flops).
"""

import numpy as np
from contextlib import ExitStack

import concourse.bass as bass
import concourse.tile as tile
from concourse import mybir
from concourse.masks import make_identity

B = 512          # batch
Z = 256          # latent dim
NCORES = 8
IB = B // NCORES   # 64 local samples per core
J = B              # pairwise j axis
P = 128            # partitions
KK = Z // 2        # 128 k-pairs (k, k+128)
CHW = 3 * 64 * 64
REC_F = IB * CHW // P   # 6144 free elems/partition of the image shard
RCH = 1024              # rec chunk (free elems per partition)
NRC = REC_F // RCH      # 6 chunks
LOG2PI = float(np.log(2.0 * np.pi))

f32 = mybir.dt.float32
bf16 = mybir.dt.bfloat16
AF = mybir.ActivationFunctionType
OP = mybir.AluOpType
AX = mybir.AxisListType




def _vmul(nc, out, a, b):
    # a*b via scalar_tensor_tensor: (a mult 1.0) mult b  (TT encoding has
    # only one sync-wait slot in walrus; TensorScalarPtr has more)
    nc.vector.scalar_tensor_tensor(out, a, 1.0, b, OP.mult, OP.mult)


def _vadd(nc, out, a, b):
    nc.vector.scalar_tensor_tensor(out, a, 0.0, b, OP.add, OP.add)


def _vcopy(nc, out, in_):
    nc.vector.tensor_scalar(out, in_, 0.0, None, OP.add)


def _split_multi_waits(nc):
    """This container's walrus accepts only ONE embedded sync-wait per
    compute/DMA instruction ("Too many sync wait commands").  Hoist extra
    waits onto same-engine NoOp carriers inserted immediately before the
    instruction — engines execute their stream in order, so this is
    semantics-preserving."""
    wid = 0
    for f in nc.m.functions:
        for blk in f.blocks:
            il = blk.instructions
            i = 0
            while i < len(il):
                ins = il[i]
                si = ins.sync_info
                tname = type(ins).__name__
                if si is not None and len(si.on_wait) > 1 and tname != "InstNoOp":
                    waits = list(si.on_wait)
                    nops = []
                    for w in waits[:-1]:
                        nop = mybir.InstNoOp(name=f"WSPLIT-{wid}", ins=[],
                                             outs=[], text_hint="wait_split")
                        wid += 1
                        nop.engine = ins.engine
                        nop.sync_info = mybir.SyncInfo(on_wait=[w], on_update=[])
                        nc.register_instruction(nop, overwrite=True)
                        nops.append(nop)
                    ins.sync_info = mybir.SyncInfo(on_wait=[waits[-1]],
                                                   on_update=list(si.on_update))
                    for j, nop in enumerate(nops):
                        il.insert(i + j, nop)
                    i += len(nops)
                i += 1
    return nc


def build_program():
    nc = bass.Bass("TRN2", target_bir_lowering=False, debug=False)

    d_mu = nc.dram_tensor("mu", [B, Z], f32, kind="ExternalInput").ap()
    d_lv = nc.dram_tensor("lv", [B, Z], f32, kind="ExternalInput").ap()
    d_lat = nc.dram_tensor("lat", [IB, Z], f32, kind="ExternalInput").ap()
    d_data = nc.dram_tensor("data", [P, REC_F], f32, kind="ExternalInput").ap()
    d_rec = nc.dram_tensor("recon", [P, REC_F], f32, kind="ExternalInput").ap()

    o_pm = nc.dram_tensor("o_pm", [P, 1], f32, kind="ExternalOutput").ap()
    o_s1 = nc.dram_tensor("o_s1", [IB, 2], f32, kind="ExternalOutput").ap()
    o_rec = nc.dram_tensor("o_rec", [P, NRC * 3], f32, kind="ExternalOutput").ap()
    o_dwkl = nc.dram_tensor("o_dwkl", [P, 2], f32, kind="ExternalOutput").ap()

    HK = KK // 2  # 64 process indices per row-group half

    with tile.TileContext(nc) as tc, ExitStack() as ctx:
        keep = ctx.enter_context(tc.tile_pool(name="keep", bufs=1))

        identity = keep.tile([P, P], bf16)
        make_identity(nc, identity)
        ones_col = keep.tile([P, 1], bf16)
        nc.gpsimd.memset(ones_col, 1.0)
        mhalf_row = keep.tile([1, IB], bf16)
        nc.gpsimd.memset(mhalf_row, -0.5)

        # transposed (k-major) coefficient tensors; dim1 = k half (k, k+128)
        WT = keep.tile([P, 2, J], bf16)
        G2T = keep.tile([P, 2, J], bf16)
        QT = keep.tile([P, 2, J], bf16)
        AT = keep.tile([P, 2, IB], bf16)
        BT = keep.tile([P, 2, IB], bf16)

        # stationary (block-diag) + moving tiles, split into two partition
        # row-groups (base 0 / base 32) so LDWEIGHTS of one group overlaps
        # the in-flight matmul of the other (different PE row_grps).
        LHS_E = keep.tile([6, HK * P], bf16)
        RHS_E = keep.tile([6, HK * J], bf16)
        LHS_Of = keep.tile([38, HK * P], bf16)
        RHS_Of = keep.tile([38, HK * J], bf16)
        LHS_O = LHS_Of[32:38]
        RHS_O = RHS_Of[32:38]

        A_red = keep.tile([P, KK], f32)     # sum_j exp(logqz_mat)
        ACCR = keep.tile([P, NRC * 3], f32)  # rec partial sums
        qvS = keep.tile([1, J], bf16)
        OS1 = keep.tile([IB, 2], f32)
        negmax = keep.tile([IB, 1], f32)

        LHSvE = LHS_E.rearrange("r (g n) -> r g n", g=HK)
        RHSvE = RHS_E.rearrange("r (g n) -> r g n", g=HK)
        LHSvO = LHS_O.rearrange("r (g n) -> r g n", g=HK)
        RHSvO = RHS_O.rearrange("r (g n) -> r g n", g=HK)

        zrow = keep.tile([1, IB], bf16)
        nc.gpsimd.memset(zrow, 0.0)

        # ---------------- prep ----------------
        with tc.tile_pool(name="prep", bufs=1) as prep:
            MU = prep.tile([P, 4, Z], f32)
            nc.sync.dma_start(MU, d_mu.rearrange("(t p) k -> p t k", p=P))
            LVt = prep.tile([P, 4, Z], f32)
            nc.sync.dma_start(LVt, d_lv.rearrange("(t p) k -> p t k", p=P))
            S0 = prep.tile([IB, Z], f32)
            nc.sync.dma_start(S0, d_lat)
            MUf = MU.rearrange("p t k -> p (t k)")
            LVf = LVt.rearrange("p t k -> p (t k)")

            # per-(j,k) coefficients
            WS = prep.tile([P, 4 * Z], f32)
            nc.scalar.activation(WS, LVf, AF.Exp)
            G2S = prep.tile([P, 4 * Z], f32)
            _vmul(nc, G2S, MUf, WS)
            QS = prep.tile([P, 4 * Z], f32)
            _vmul(nc, QS, MUf, G2S)
            nc.vector.scalar_tensor_tensor(QS, QS, LOG2PI, LVf, OP.add, OP.add)
            wb = prep.tile([P, 4, Z], bf16)
            _vcopy(nc, wb.rearrange("p t k -> p (t k)"), WS)
            g2b = prep.tile([P, 4, Z], bf16)
            _vcopy(nc, g2b.rearrange("p t k -> p (t k)"), G2S)
            qb = prep.tile([P, 4, Z], bf16)
            _vcopy(nc, qb.rearrange("p t k -> p (t k)"), QS)

            # per-(i,k) coefficients
            SSQ = prep.tile([IB, Z], f32)
            _vmul(nc, SSQ, S0, S0)
            ab = prep.tile([IB, Z], bf16)
            nc.vector.tensor_scalar_mul(ab, SSQ, -0.5)
            sb = prep.tile([IB, Z], bf16)
            _vcopy(nc, sb, S0)

            # dimension-wise KL partials: sum exp(mu^2+lv) and sum lv
            DW = prep.tile([P, 2], f32)
            MSQ = prep.tile([P, 4 * Z], f32)
            _vmul(nc, MSQ, MUf, MUf)
            _vadd(nc, MSQ, MSQ, LVf)
            nc.scalar.activation(MSQ, MSQ, AF.Exp, accum_out=DW[:, 0:1])
            nc.vector.tensor_scalar(MSQ, LVf, 1.0, None, OP.mult, OP.add,
                                    accum_out=DW[:, 1:2])
            nc.sync.dma_start(o_dwkl, DW)

            # transpose to k-major via TensorE
            with tc.tile_pool(name="tpsum", bufs=3, space="PSUM") as tpsum:
                for srcb, dstT in ((wb, WT), (g2b, G2T), (qb, QT)):
                    for kh in range(2):
                        for jt in range(4):
                            pt = tpsum.tile([P, P], bf16, tag="tp")
                            nc.tensor.transpose(
                                pt, srcb[:, jt, kh * 128:(kh + 1) * 128], identity)
                            _vcopy(nc, dstT[:, kh, jt * 128:(jt + 1) * 128], pt)
                for srcb, dstT in ((ab, AT), (sb, BT)):
                    for kh in range(2):
                        pt = tpsum.tile([P, P], bf16, tag="tp")
                        nc.tensor.transpose(
                            pt[:, 0:IB], srcb[:, kh * 128:(kh + 1) * 128],
                            identity[0:IB, 0:IB])
                        _vcopy(nc, dstT[:, kh, :], pt[:, 0:IB])

            # zero the unused halves of the block-diagonal stationary
            # tiles: engine memset for rows at a legal base partition, a
            # broadcast DMA for the rest.
            zbcast = bass.AP(tensor=zrow.tensor, offset=zrow.offset,
                             ap=[list(zrow.ap[0]), [0, HK], [1, IB]])
            nc.gpsimd.memset(LHSvE[0:3, :, IB:P], 0.0)
            nc.sync.dma_start(LHSvE[3:4, :, 0:IB], zbcast)
            nc.sync.dma_start(LHSvE[4:5, :, 0:IB], zbcast)
            nc.sync.dma_start(LHSvE[5:6, :, 0:IB], zbcast)
            nc.gpsimd.memset(LHSvO[0:3, :, IB:P], 0.0)
            nc.sync.dma_start(LHSvO[3:4, :, 0:IB], zbcast)
            nc.sync.dma_start(LHSvO[4:5, :, 0:IB], zbcast)
            nc.sync.dma_start(LHSvO[5:6, :, 0:IB], zbcast)

            # gather into interleaved buffers; spread across DMA queues.
            # process index m: even -> E group (WT partition m//2),
            #                  odd  -> O group (WT partition 64 + m//2)
            dq = [nc.sync, nc.gpsimd]
            qi = 0
            for dst, psl in ((RHSvE, slice(0, HK)), (RHSvO, slice(HK, P))):
                for r, (src, kh) in enumerate(
                        ((WT, 0), (G2T, 0), (QT, 0), (WT, 1), (G2T, 1), (QT, 1))):
                    dq[qi % len(dq)].dma_start(dst[r:r + 1], src[psl, kh, :])
                    qi += 1
            for dst, psl in ((LHSvE, slice(0, HK)), (LHSvO, slice(HK, P))):
                dq[qi % len(dq)].dma_start(dst[0:1, :, 0:IB], AT[psl, 0, :]); qi += 1
                dq[qi % len(dq)].dma_start(dst[1:2, :, 0:IB], BT[psl, 0, :]); qi += 1
                dq[qi % len(dq)].dma_start(dst[3:4, :, IB:P], AT[psl, 1, :]); qi += 1
                dq[qi % len(dq)].dma_start(dst[4:5, :, IB:P], BT[psl, 1, :]); qi += 1
            mhalf_bcast = bass.AP(tensor=mhalf_row.tensor, offset=mhalf_row.offset,
                                  ap=[list(mhalf_row.ap[0]), [0, HK], [1, IB]])
            for dst in (LHSvE, LHSvO):
                dq[qi % len(dq)].dma_start(dst[2:3, :, 0:IB], mhalf_bcast); qi += 1
                dq[qi % len(dq)].dma_start(dst[5:6, :, IB:P], mhalf_bcast); qi += 1

        # ---------------- main pairwise loop (rec BCE interleaved) --------
        NG = KK // 4
        rec_at = {2 + 5 * c: c for c in range(NRC)}  # group idx -> rec chunk
        with tc.tile_pool(name="mpsum", bufs=2, space="PSUM") as mp, \
                tc.tile_pool(name="epool", bufs=2) as ep, \
                tc.tile_pool(name="rpool", bufs=2) as rp, \
                tc.tile_pool(name="rpool1", bufs=1) as rp1:
            for g in range(NG):
                T4 = mp.tile([P, 4, J], f32)
                for c in range(4):
                    m = 4 * g + c
                    h = m // 2
                    if m % 2 == 0:
                        lhs, rhs = LHSvE[:, h, :], RHSvE[:, h, :]
                    else:
                        lhs, rhs = LHSvO[:, h, :], RHSvO[:, h, :]
                    nc.tensor.matmul(T4[:, c, :], lhs, rhs, start=True, stop=True)
                E4 = ep.tile([P, 4, J], bf16)
                nc.scalar.activation(E4.rearrange("p c j -> p (c j)"),
                                     T4.rearrange("p c j -> p (c j)"), AF.Exp)
                # halving-tree sum over j (bf16 2x tensor-tensor adds), then a
                # single fp32 reduce of the 8-wide stumps into A_red columns
                hh = J // 2
                while hh >= 16:
                    nc.vector.tensor_add(E4[:, :, 0:hh], E4[:, :, 0:hh],
                                         E4[:, :, hh:2 * hh])
                    hh //= 2
                nc.vector.tensor_reduce(A_red[:, 4 * g:4 * g + 4], E4[:, :, 0:16],
                                        axis=AX.X, op=OP.add)

                if g in rec_at:
                    ch = rec_at[g]
                    sl = slice(ch * RCH, (ch + 1) * RCH)
                    DD = rp.tile([P, RCH], f32)
                    nc.gpsimd.dma_start(DD, d_data[:, sl])
                    RR = rp.tile([P, RCH], f32)
                    nc.gpsimd.dma_start(RR, d_rec[:, sl])
                    DDb = rp1.tile([P, RCH], bf16)
                    _vcopy(nc, DDb, DD)
                    LR = rp1.tile([P, RCH], bf16)
                    nc.scalar.activation(LR, RR, AF.Ln)
                    L1R = rp1.tile([P, RCH], bf16)
                    nc.scalar.activation(L1R, RR, AF.Ln, bias=1.0, scale=-1.0,
                                         accum_out=ACCR[:, 3 * ch + 1:3 * ch + 2])
                    SC = rp1.tile([P, RCH], bf16)
                    nc.vector.scalar_tensor_tensor(
                        SC, DDb, 1.0, LR, OP.mult, OP.mult,
                        accum_out=ACCR[:, 3 * ch:3 * ch + 1])
                    nc.vector.scalar_tensor_tensor(
                        SC, DDb, -1.0, L1R, OP.mult, OP.mult,
                        accum_out=ACCR[:, 3 * ch + 2:3 * ch + 3])
        nc.sync.dma_start(o_rec, ACCR)

        LG = keep.tile([P, KK], f32)
        nc.scalar.activation(LG, A_red, AF.Ln)
        PM = keep.tile([P, 1], f32)
        nc.vector.reduce_sum(PM, LG, axis=AX.X)
        nc.sync.dma_start(o_pm, PM)

        # ---------------- logqz path (S1 = sum_k logqz_mat) ----------------
        with tc.tile_pool(name="s1psum", bufs=1, space="PSUM") as s1p, \
                tc.tile_pool(name="s1sb", bufs=1) as s1sb:
            qpv = s1p.tile([1, J], f32)
            nc.tensor.matmul(qpv, ones_col, QT[:, 0, :], start=True, stop=False)
            nc.tensor.matmul(qpv, ones_col, QT[:, 1, :], start=False, stop=True)
            _vcopy(nc, qvS, qpv)

            S1 = s1p.tile([IB, J], f32)
            nc.tensor.matmul(S1, AT[:, 0, :], WT[:, 0, :], start=True, stop=False)
            nc.tensor.matmul(S1, BT[:, 0, :], G2T[:, 0, :], start=False, stop=False)
            nc.tensor.matmul(S1, AT[:, 1, :], WT[:, 1, :], start=False, stop=False)
            nc.tensor.matmul(S1, BT[:, 1, :], G2T[:, 1, :], start=False, stop=False)
            nc.tensor.matmul(S1, mhalf_row, qvS, start=False, stop=True)

            nc.vector.tensor_reduce(negmax, S1, axis=AX.X, op=OP.max, negate=True)
            es = s1sb.tile([IB, J], bf16)
            nc.scalar.activation(es, S1, AF.Exp, bias=negmax, scale=1.0,
                                 accum_out=OS1[:, 1:2])
            _vcopy(nc, OS1[:, 0:1], negmax)
            nc.sync.dma_start(o_s1, OS1)

    return _split_multi_waits(nc)


def make_in_maps(data, recon, lat, mu, lv):
    in_maps = []
    for c in range(NCORES):
        sl = slice(c * IB, (c + 1) * IB)
        in_maps.append({
            "mu": np.ascontiguousarray(mu, np.float32),
            "lv": np.ascontiguousarray(lv, np.float32),
            "lat": np.ascontiguousarray(lat[sl], np.float32),
            "data": np.ascontiguousarray(
                np.asarray(data[sl], np.float32).reshape(P, REC_F)),
            "recon": np.ascontiguousarray(
                np.asarray(recon[sl], np.float32).reshape(P, REC_F)),
        })
    return in_maps


def combine(results, dataset_size):
    """results: list of 8 dicts with per-core output tensors."""
    log_norm = float(np.log(np.float32(B)) + np.log(np.float32(float(dataset_size))))

    rec_sum = sum(r["o_rec"].astype(np.float64).sum() for r in results)
    rec_loss = -rec_sum / B

    dw = results[0]["o_dwkl"].astype(np.float64)
    dwkl = (0.5 * dw[:, 0].sum() - 0.5 * dw[:, 1].sum() - 0.5 * B * Z) / B

    tc_total = 0.0
    for r in results:
        pmh = r["o_pm"].astype(np.float64).ravel()
        pm = pmh[:IB] + pmh[IB:]
        prodmarg = pm - Z * log_norm
        s1 = r["o_s1"].astype(np.float64)
        lq = (-s1[:, 0]) + np.log(s1[:, 1]) - log_norm
        tc_total += (lq - prodmarg).sum()
    tc_loss = tc_total / B

    return np.array(rec_loss + tc_loss + dwkl, dtype=np.float32)


def run_on_hw(inputs, trace=False):
    from concourse.bass_utils import run_bass_kernel_spmd

    nc = build_program()
    in_maps = make_in_maps(inputs["data"], inputs["recon_batch"],
                           inputs["latent_sample"], inputs["mu"],
                           inputs["logvar"])
    br = run_bass_kernel_spmd(nc, in_maps, list(range(NCORES)), trace=trace)
    elbo = combine(br.results, inputs["dataset_size"])
    return elbo, br


def kernel(**inputs):
    elbo, _ = run_on_hw(inputs, trace=False)
    return elbo


# revision 15
# speedup vs baseline: 1.3091x; 1.0333x over previous
"""Trainium2 Bass kernel for nn_BatchTCLoss (beta-TCVAE ELBO loss).

Strategy (8 NeuronCores, data-parallel over the sample axis i):
  - Each core owns 64 of the 512 latent rows (and the matching 64 images for
    the BCE term); mu/logvar are replicated.
  - logqz_mat[i,j,k] = -0.5*((s_ik-mu_jk)^2*exp(lv_jk) + lv_jk + LOG2PI)
    expands as a_ik*w_jk + b_ik*g2_jk + c*q_jk with
      a = -0.5*s^2, b = s, c = -0.5
      w = exp(lv), g2 = mu*w, q = mu^2*w + lv + LOG2PI
    so each (i, k)-slice over all j is a rank-3 matmul.  Two k-slices are
    packed per 128x512 PSUM tile via a 6-row block-diagonal lhsT, giving
    full-width TensorE + ScalarE tiles.
  - Per-(i,k) logsumexp over j: exp on ScalarE (values are <= exp(-0.69), no
    max-subtraction needed), row-sum fused into VectorE tensor_scalar
    accumulators, log at the end.
  - logqz: S1[i,j] = sum_k logqz_mat via 5 accumulated matmuls, then a
    max-stabilized exp-sum on one 64x512 tile.
  - BCE + dimension-wise KL are streamed elementwise reductions.
  - Each core emits tiny per-core partial tensors; the host combines them
    (the final reduction is O(1000) flops).
"""

import numpy as np
from contextlib import ExitStack

import concourse.bass as bass
import concourse.tile as tile
from concourse import mybir
from concourse.masks import make_identity

B = 512          # batch
Z = 256          # latent dim
NCORES = 8
IB = B // NCORES   # 64 local samples per core
J = B              # pairwise j axis
P = 128            # partitions
KK = Z // 2        # 128 k-pairs (k, k+128)
CHW = 3 * 64 * 64
REC_F = IB * CHW // P   # 6144 free elems/partition of the image shard
RCH = 1024              # rec chunk (free elems per partition)
NRC = REC_F // RCH      # 6 chunks
LOG2PI = float(np.log(2.0 * np.pi))

f32 = mybir.dt.float32
bf16 = mybir.dt.bfloat16
AF = mybir.ActivationFunctionType
OP = mybir.AluOpType
AX = mybir.AxisListType




def _vmul(nc, out, a, b):
    # a*b via scalar_tensor_tensor: (a mult 1.0) mult b  (TT encoding has
    # only one sync-wait slot in walrus; TensorScalarPtr has more)
    nc.vector.scalar_tensor_tensor(out, a, 1.0, b, OP.mult, OP.mult)


def _vadd(nc, out, a, b):
    nc.vector.scalar_tensor_tensor(out, a, 0.0, b, OP.add, OP.add)


def _vcopy(nc, out, in_):
    nc.vector.tensor_scalar(out, in_, 0.0, None, OP.add)


def _split_multi_waits(nc):
    """This container's walrus accepts only ONE embedded sync-wait per
    compute/DMA instruction ("Too many sync wait commands").  Hoist extra
    waits onto same-engine NoOp carriers inserted immediately before the
    instruction — engines execute their stream in order, so this is
    semantics-preserving."""
    wid = 0
    for f in nc.m.functions:
        for blk in f.blocks:
            il = blk.instructions
            i = 0
            while i < len(il):
                ins = il[i]
                si = ins.sync_info
                tname = type(ins).__name__
                if si is not None and len(si.on_wait) > 1 and tname != "InstNoOp":
                    waits = list(si.on_wait)
                    nops = []
                    for w in waits[:-1]:
                        nop = mybir.InstNoOp(name=f"WSPLIT-{wid}", ins=[],
                                             outs=[], text_hint="wait_split")
                        wid += 1
                        nop.engine = ins.engine
                        nop.sync_info = mybir.SyncInfo(on_wait=[w], on_update=[])
                        nc.register_instruction(nop, overwrite=True)
                        nops.append(nop)
                    ins.sync_info = mybir.SyncInfo(on_wait=[waits[-1]],
                                                   on_update=list(si.on_update))
                    for j, nop in enumerate(nops):
                        il.insert(i + j, nop)
                    i += len(nops)
                i += 1
    return nc


def build_program():
    nc = bass.Bass("TRN2", target_bir_lowering=False, debug=False)

    d_mu = nc.dram_tensor("mu", [B, Z], f32, kind="ExternalInput").ap()
    d_lv = nc.dram_tensor("lv", [B, Z], f32, kind="ExternalInput").ap()
    d_lat = nc.dram_tensor("lat", [IB, Z], f32, kind="ExternalInput").ap()
    d_data = nc.dram_tensor("data", [P, REC_F], f32, kind="ExternalInput").ap()
    d_rec = nc.dram_tensor("recon", [P, REC_F], f32, kind="ExternalInput").ap()

    o_pm = nc.dram_tensor("o_pm", [P, 1], f32, kind="ExternalOutput").ap()
    o_s1 = nc.dram_tensor("o_s1", [IB, 2], f32, kind="ExternalOutput").ap()
    o_rec = nc.dram_tensor("o_rec", [P, NRC * 3], f32, kind="ExternalOutput").ap()
    o_dwkl = nc.dram_tensor("o_dwkl", [P, 2], f32, kind="ExternalOutput").ap()

    HK = KK // 2  # 64 process indices per row-group half

    with tile.TileContext(nc) as tc, ExitStack() as ctx:
        keep = ctx.enter_context(tc.tile_pool(name="keep", bufs=1))

        identity = keep.tile([P, P], bf16)
        make_identity(nc, identity)
        ones_col = keep.tile([P, 1], bf16)
        nc.gpsimd.memset(ones_col, 1.0)
        mhalf_row = keep.tile([1, IB], bf16)
        nc.gpsimd.memset(mhalf_row, -0.5)

        # transposed (k-major) coefficient tensors; dim1 = k half (k, k+128)
        WT = keep.tile([P, 2, J], bf16)
        G2T = keep.tile([P, 2, J], bf16)
        QT = keep.tile([P, 2, J], bf16)
        AT = keep.tile([P, 2, IB], bf16)
        BT = keep.tile([P, 2, IB], bf16)

        # stationary (block-diag) + moving tiles, split into two partition
        # row-groups (base 0 / base 32) so LDWEIGHTS of one group overlaps
        # the in-flight matmul of the other (different PE row_grps).
        LHS_E = keep.tile([6, HK * P], bf16)
        RHS_E = keep.tile([6, HK * J], bf16)
        LHS_Of = keep.tile([38, HK * P], bf16)
        RHS_Of = keep.tile([38, HK * J], bf16)
        LHS_O = LHS_Of[32:38]
        RHS_O = RHS_Of[32:38]

        A_red = keep.tile([P, KK], f32)     # sum_j exp(logqz_mat)
        ACCR = keep.tile([P, NRC * 3], f32)  # rec partial sums
        qvS = keep.tile([1, J], bf16)
        OS1 = keep.tile([IB, 2], f32)
        negmax = keep.tile([IB, 1], f32)

        LHSvE = LHS_E.rearrange("r (g n) -> r g n", g=HK)
        RHSvE = RHS_E.rearrange("r (g n) -> r g n", g=HK)
        LHSvO = LHS_O.rearrange("r (g n) -> r g n", g=HK)
        RHSvO = RHS_O.rearrange("r (g n) -> r g n", g=HK)

        zrow = keep.tile([1, IB], bf16)
        nc.gpsimd.memset(zrow, 0.0)

        # ---------------- prep ----------------
        with tc.tile_pool(name="prep", bufs=1) as prep:
            MU = prep.tile([P, 4, Z], f32)
            nc.sync.dma_start(MU, d_mu.rearrange("(t p) k -> p t k", p=P))
            LVt = prep.tile([P, 4, Z], f32)
            nc.sync.dma_start(LVt, d_lv.rearrange("(t p) k -> p t k", p=P))
            S0 = prep.tile([IB, Z], f32)
            nc.sync.dma_start(S0, d_lat)
            MUf = MU.rearrange("p t k -> p (t k)")
            LVf = LVt.rearrange("p t k -> p (t k)")

            # per-(j,k) coefficients
            WS = prep.tile([P, 4 * Z], f32)
            nc.scalar.activation(WS, LVf, AF.Exp)
            G2S = prep.tile([P, 4 * Z], f32)
            _vmul(nc, G2S, MUf, WS)
            QS = prep.tile([P, 4 * Z], f32)
            _vmul(nc, QS, MUf, G2S)
            nc.vector.scalar_tensor_tensor(QS, QS, LOG2PI, LVf, OP.add, OP.add)
            wb = prep.tile([P, 4, Z], bf16)
            _vcopy(nc, wb.rearrange("p t k -> p (t k)"), WS)
            g2b = prep.tile([P, 4, Z], bf16)
            _vcopy(nc, g2b.rearrange("p t k -> p (t k)"), G2S)
            qb = prep.tile([P, 4, Z], bf16)
            _vcopy(nc, qb.rearrange("p t k -> p (t k)"), QS)

            # per-(i,k) coefficients
            SSQ = prep.tile([IB, Z], f32)
            _vmul(nc, SSQ, S0, S0)
            ab = prep.tile([IB, Z], bf16)
            nc.vector.tensor_scalar_mul(ab, SSQ, -0.5)
            sb = prep.tile([IB, Z], bf16)
            _vcopy(nc, sb, S0)

            # dimension-wise KL partials: sum exp(mu^2+lv) and sum lv
            DW = prep.tile([P, 2], f32)
            MSQ = prep.tile([P, 4 * Z], f32)
            _vmul(nc, MSQ, MUf, MUf)
            _vadd(nc, MSQ, MSQ, LVf)
            nc.scalar.activation(MSQ, MSQ, AF.Exp, accum_out=DW[:, 0:1])
            nc.vector.tensor_scalar(MSQ, LVf, 1.0, None, OP.mult, OP.add,
                                    accum_out=DW[:, 1:2])
            nc.sync.dma_start(o_dwkl, DW)

            # transpose to k-major via TensorE
            with tc.tile_pool(name="tpsum", bufs=3, space="PSUM") as tpsum:
                for srcb, dstT in ((wb, WT), (g2b, G2T), (qb, QT)):
                    for kh in range(2):
                        for jt in range(4):
                            pt = tpsum.tile([P, P], bf16, tag="tp")
                            nc.tensor.transpose(
                                pt, srcb[:, jt, kh * 128:(kh + 1) * 128], identity)
                            _vcopy(nc, dstT[:, kh, jt * 128:(jt + 1) * 128], pt)
                for srcb, dstT in ((ab, AT), (sb, BT)):
                    for kh in range(2):
                        pt = tpsum.tile([P, P], bf16, tag="tp")
                        nc.tensor.transpose(
                            pt[:, 0:IB], srcb[:, kh * 128:(kh + 1) * 128],
                            identity[0:IB, 0:IB])
                        _vcopy(nc, dstT[:, kh, :], pt[:, 0:IB])

            # zero the unused halves of the block-diagonal stationary
            # tiles: engine memset for rows at a legal base partition, a
            # broadcast DMA for the rest.
            zbcast = bass.AP(tensor=zrow.tensor, offset=zrow.offset,
                             ap=[list(zrow.ap[0]), [0, HK], [1, IB]])
            nc.gpsimd.memset(LHSvE[0:3, :, IB:P], 0.0)
            nc.sync.dma_start(LHSvE[3:4, :, 0:IB], zbcast)
            nc.sync.dma_start(LHSvE[4:5, :, 0:IB], zbcast)
            nc.sync.dma_start(LHSvE[5:6, :, 0:IB], zbcast)
            nc.gpsimd.memset(LHSvO[0:3, :, IB:P], 0.0)
            nc.sync.dma_start(LHSvO[3:4, :, 0:IB], zbcast)
            nc.sync.dma_start(LHSvO[4:5, :, 0:IB], zbcast)
            nc.sync.dma_start(LHSvO[5:6, :, 0:IB], zbcast)

            # gather into interleaved buffers; spread across DMA queues.
            # process index m: even -> E group (WT partition m//2),
            #                  odd  -> O group (WT partition 64 + m//2)
            dq = [nc.sync, nc.gpsimd]
            qi = 0
            for dst, psl in ((RHSvE, slice(0, HK)), (RHSvO, slice(HK, P))):
                for r, (src, kh) in enumerate(
                        ((WT, 0), (G2T, 0), (QT, 0), (WT, 1), (G2T, 1), (QT, 1))):
                    dq[qi % len(dq)].dma_start(dst[r:r + 1], src[psl, kh, :])
                    qi += 1
            for dst, psl in ((LHSvE, slice(0, HK)), (LHSvO, slice(HK, P))):
                dq[qi % len(dq)].dma_start(dst[0:1, :, 0:IB], AT[psl, 0, :]); qi += 1
                dq[qi % len(dq)].dma_start(dst[1:2, :, 0:IB], BT[psl, 0, :]); qi += 1
                dq[qi % len(dq)].dma_start(dst[3:4, :, IB:P], AT[psl, 1, :]); qi += 1
                dq[qi % len(dq)].dma_start(dst[4:5, :, IB:P], BT[psl, 1, :]); qi += 1
            mhalf_bcast = bass.AP(tensor=mhalf_row.tensor, offset=mhalf_row.offset,
                                  ap=[list(mhalf_row.ap[0]), [0, HK], [1, IB]])
            for dst in (LHSvE, LHSvO):
                dq[qi % len(dq)].dma_start(dst[2:3, :, 0:IB], mhalf_bcast); qi += 1
                gate = dq[qi % len(dq)].dma_start(dst[5:6, :, IB:P], mhalf_bcast)
                qi += 1

        # ---------------- logqz path (S1 = sum_k logqz_mat) ----------------
        with tc.tile_pool(name="s1psum", bufs=1, space="PSUM") as s1p, \
                tc.tile_pool(name="s1sb", bufs=1) as s1sb:
            qpv = s1p.tile([1, J], f32)
            nc.tensor.matmul(qpv, ones_col, QT[:, 0, :], start=True, stop=False)
            nc.tensor.matmul(qpv, ones_col, QT[:, 1, :], start=False, stop=True)
            _vcopy(nc, qvS, qpv)

            S1 = s1p.tile([IB, J], f32)
            nc.tensor.matmul(S1, AT[:, 0, :], WT[:, 0, :], start=True, stop=False)
            nc.tensor.matmul(S1, BT[:, 0, :], G2T[:, 0, :], start=False, stop=False)
            nc.tensor.matmul(S1, AT[:, 1, :], WT[:, 1, :], start=False, stop=False)
            nc.tensor.matmul(S1, BT[:, 1, :], G2T[:, 1, :], start=False, stop=False)
            nc.tensor.matmul(S1, mhalf_row, qvS, start=False, stop=True)

            nc.vector.tensor_reduce(negmax, S1, axis=AX.X, op=OP.max, negate=True)
            es = s1sb.tile([IB, J], bf16)
            nc.scalar.activation(es, S1, AF.Exp, bias=negmax, scale=1.0,
                                 accum_out=OS1[:, 1:2])
            _vcopy(nc, OS1[:, 0:1], negmax)
            nc.sync.dma_start(o_s1, OS1)

        # ---------------- main pairwise loop (rec BCE interleaved) --------
        NG = KK // 4
        rec_at = {2 + 5 * c: c for c in range(NRC)}  # group idx -> rec chunk
        with tc.tile_pool(name="mpsum", bufs=2, space="PSUM") as mp, \
                tc.tile_pool(name="epool", bufs=2) as ep, \
                tc.tile_pool(name="rpool", bufs=2) as rp, \
                tc.tile_pool(name="rpool1", bufs=1) as rp1:
            for g in range(NG):
                T4 = mp.tile([P, 4, J], f32)
                for c in range(4):
                    m = 4 * g + c
                    h = m // 2
                    if m % 2 == 0:
                        lhs, rhs = LHSvE[:, h, :], RHSvE[:, h, :]
                    else:
                        lhs, rhs = LHSvO[:, h, :], RHSvO[:, h, :]
                    nc.tensor.matmul(T4[:, c, :], lhs, rhs, start=True, stop=True)
                E4 = ep.tile([P, 4, J], bf16)
                nc.scalar.activation(E4.rearrange("p c j -> p (c j)"),
                                     T4.rearrange("p c j -> p (c j)"), AF.Exp)
                # halving-tree sum over j (bf16 2x tensor-tensor adds), then a
                # single fp32 reduce of the 8-wide stumps into A_red columns
                hh = J // 2
                while hh >= 16:
                    nc.vector.tensor_add(E4[:, :, 0:hh], E4[:, :, 0:hh],
                                         E4[:, :, hh:2 * hh])
                    hh //= 2
                nc.vector.tensor_reduce(A_red[:, 4 * g:4 * g + 4], E4[:, :, 0:16],
                                        axis=AX.X, op=OP.add)

                if g in rec_at:
                    ch = rec_at[g]
                    sl = slice(ch * RCH, (ch + 1) * RCH)
                    DD = rp.tile([P, RCH], f32)
                    dd_i = nc.gpsimd.dma_start(DD, d_data[:, sl])
                    tile.add_dep_helper(dd_i.ins, gate.ins, sync=True,
                                        reason="rec prefetch after gathers")
                    RR = rp.tile([P, RCH], f32)
                    rr_i = nc.gpsimd.dma_start(RR, d_rec[:, sl])
                    tile.add_dep_helper(rr_i.ins, gate.ins, sync=True,
                                        reason="rec prefetch after gathers")
                    DDb = rp1.tile([P, RCH], bf16)
                    _vcopy(nc, DDb, DD)
                    LR = rp1.tile([P, RCH], bf16)
                    nc.scalar.activation(LR, RR, AF.Ln)
                    L1R = rp1.tile([P, RCH], bf16)
                    nc.scalar.activation(L1R, RR, AF.Ln, bias=1.0, scale=-1.0,
                                         accum_out=ACCR[:, 3 * ch + 1:3 * ch + 2])
                    SC = rp1.tile([P, RCH], bf16)
                    nc.vector.scalar_tensor_tensor(
                        SC, DDb, 1.0, LR, OP.mult, OP.mult,
                        accum_out=ACCR[:, 3 * ch:3 * ch + 1])
                    nc.vector.scalar_tensor_tensor(
                        SC, DDb, -1.0, L1R, OP.mult, OP.mult,
                        accum_out=ACCR[:, 3 * ch + 2:3 * ch + 3])
        nc.sync.dma_start(o_rec, ACCR)

        LG = keep.tile([P, KK], f32)
        nc.scalar.activation(LG, A_red, AF.Ln)
        PM = keep.tile([P, 1], f32)
        nc.vector.reduce_sum(PM, LG, axis=AX.X)
        nc.sync.dma_start(o_pm, PM)

    return _split_multi_waits(nc)


def make_in_maps(data, recon, lat, mu, lv):
    in_maps = []
    for c in range(NCORES):
        sl = slice(c * IB, (c + 1) * IB)
        in_maps.append({
            "mu": np.ascontiguousarray(mu, np.float32),
            "lv": np.ascontiguousarray(lv, np.float32),
            "lat": np.ascontiguousarray(lat[sl], np.float32),
            "data": np.ascontiguousarray(
                np.asarray(data[sl], np.float32).reshape(P, REC_F)),
            "recon": np.ascontiguousarray(
                np.asarray(recon[sl], np.float32).reshape(P, REC_F)),
        })
    return in_maps


def combine(results, dataset_size):
    """results: list of 8 dicts with per-core output tensors."""
    log_norm = float(np.log(np.float32(B)) + np.log(np.float32(float(dataset_size))))

    rec_sum = sum(r["o_rec"].astype(np.float64).sum() for r in results)
    rec_loss = -rec_sum / B

    dw = results[0]["o_dwkl"].astype(np.float64)
    dwkl = (0.5 * dw[:, 0].sum() - 0.5 * dw[:, 1].sum() - 0.5 * B * Z) / B

    tc_total = 0.0
    for r in results:
        pmh = r["o_pm"].astype(np.float64).ravel()
        pm = pmh[:IB] + pmh[IB:]
        prodmarg = pm - Z * log_norm
        s1 = r["o_s1"].astype(np.float64)
        lq = (-s1[:, 0]) + np.log(s1[:, 1]) - log_norm
        tc_total += (lq - prodmarg).sum()
    tc_loss = tc_total / B

    return np.array(rec_loss + tc_loss + dwkl, dtype=np.float32)


def run_on_hw(inputs, trace=False):
    from concourse.bass_utils import run_bass_kernel_spmd

    nc = build_program()
    in_maps = make_in_maps(inputs["data"], inputs["recon_batch"],
                           inputs["latent_sample"], inputs["mu"],
                           inputs["logvar"])
    br = run_bass_kernel_spmd(nc, in_maps, list(range(NCORES)), trace=trace)
    elbo = combine(br.results, inputs["dataset_size"])
    return elbo, br


def kernel(**inputs):
    elbo, _ = run_on_hw(inputs, trace=False)
    return elbo


# revision 18
# speedup vs baseline: 1.3634x; 1.0415x over previous
"""Trainium2 Bass kernel for nn_BatchTCLoss (beta-TCVAE ELBO loss).

Strategy (8 NeuronCores, data-parallel over the sample axis i):
  - Each core owns 64 of the 512 latent rows (and the matching 64 images for
    the BCE term); mu/logvar are replicated.
  - logqz_mat[i,j,k] = -0.5*((s_ik-mu_jk)^2*exp(lv_jk) + lv_jk + LOG2PI)
    expands as a_ik*w_jk + b_ik*g2_jk + c*q_jk with
      a = -0.5*s^2, b = s, c = -0.5
      w = exp(lv), g2 = mu*w, q = mu^2*w + lv + LOG2PI
    so each (i, k)-slice over all j is a rank-3 matmul.  Two k-slices are
    packed per 128x512 PSUM tile via a 6-row block-diagonal lhsT, giving
    full-width TensorE + ScalarE tiles.
  - Per-(i,k) logsumexp over j: exp on ScalarE (values are <= exp(-0.69), no
    max-subtraction needed), row-sum fused into VectorE tensor_scalar
    accumulators, log at the end.
  - logqz: S1[i,j] = sum_k logqz_mat via 5 accumulated matmuls, then a
    max-stabilized exp-sum on one 64x512 tile.
  - BCE + dimension-wise KL are streamed elementwise reductions.
  - Each core emits tiny per-core partial tensors; the host combines them
    (the final reduction is O(1000) flops).
"""

import numpy as np
from contextlib import ExitStack

import concourse.bass as bass
import concourse.tile as tile
from concourse import mybir
from concourse.masks import make_identity

B = 512          # batch
Z = 256          # latent dim
NCORES = 8
IB = B // NCORES   # 64 local samples per core
J = B              # pairwise j axis
P = 128            # partitions
KK = Z // 2        # 128 k-pairs (k, k+128)
CHW = 3 * 64 * 64
REC_F = IB * CHW // P   # 6144 free elems/partition of the image shard
RCH = 1024              # rec chunk (free elems per partition)
NRC = REC_F // RCH      # 6 chunks
LOG2PI = float(np.log(2.0 * np.pi))

f32 = mybir.dt.float32
bf16 = mybir.dt.bfloat16
AF = mybir.ActivationFunctionType
OP = mybir.AluOpType
AX = mybir.AxisListType




def _vmul(nc, out, a, b):
    # a*b via scalar_tensor_tensor: (a mult 1.0) mult b  (TT encoding has
    # only one sync-wait slot in walrus; TensorScalarPtr has more)
    nc.vector.scalar_tensor_tensor(out, a, 1.0, b, OP.mult, OP.mult)


def _vadd(nc, out, a, b):
    nc.vector.scalar_tensor_tensor(out, a, 0.0, b, OP.add, OP.add)


def _vcopy(nc, out, in_):
    nc.vector.tensor_scalar(out, in_, 0.0, None, OP.add)


def _split_multi_waits(nc):
    """This container's walrus accepts only ONE embedded sync-wait per
    compute/DMA instruction ("Too many sync wait commands").  Hoist extra
    waits onto same-engine NoOp carriers inserted immediately before the
    instruction — engines execute their stream in order, so this is
    semantics-preserving."""
    wid = 0
    for f in nc.m.functions:
        for blk in f.blocks:
            il = blk.instructions
            i = 0
            while i < len(il):
                ins = il[i]
                si = ins.sync_info
                tname = type(ins).__name__
                if si is not None and len(si.on_wait) > 1 and tname != "InstNoOp":
                    waits = list(si.on_wait)
                    nops = []
                    for w in waits[:-1]:
                        nop = mybir.InstNoOp(name=f"WSPLIT-{wid}", ins=[],
                                             outs=[], text_hint="wait_split")
                        wid += 1
                        nop.engine = ins.engine
                        nop.sync_info = mybir.SyncInfo(on_wait=[w], on_update=[])
                        nc.register_instruction(nop, overwrite=True)
                        nops.append(nop)
                    ins.sync_info = mybir.SyncInfo(on_wait=[waits[-1]],
                                                   on_update=list(si.on_update))
                    for j, nop in enumerate(nops):
                        il.insert(i + j, nop)
                    i += len(nops)
                i += 1
    return nc


def build_program():
    nc = bass.Bass("TRN2", target_bir_lowering=False, debug=False)

    # host supplies k-major (transposed) copies of mu/logvar/latent —
    # pure layout work, part of sharding
    d_muT = nc.dram_tensor("muT", [Z, B], f32, kind="ExternalInput").ap()
    d_lvT = nc.dram_tensor("lvT", [Z, B], f32, kind="ExternalInput").ap()
    d_latT = nc.dram_tensor("latT", [Z, IB], f32, kind="ExternalInput").ap()
    d_data = nc.dram_tensor("data", [P, REC_F], f32, kind="ExternalInput").ap()
    d_rec = nc.dram_tensor("recon", [P, REC_F], f32, kind="ExternalInput").ap()

    o_pm = nc.dram_tensor("o_pm", [P, 1], f32, kind="ExternalOutput").ap()
    o_s1 = nc.dram_tensor("o_s1", [IB, 2], f32, kind="ExternalOutput").ap()
    o_rec = nc.dram_tensor("o_rec", [P, NRC * 3], f32, kind="ExternalOutput").ap()
    o_dwkl = nc.dram_tensor("o_dwkl", [P, 2], f32, kind="ExternalOutput").ap()

    HK = KK // 2   # 64 process indices per row-group half
    NCH = 4        # gather chunks per half
    CHB = HK // NCH  # 16 kk-blocks per chunk

    with tile.TileContext(nc) as tc, ExitStack() as ctx:
        keep = ctx.enter_context(tc.tile_pool(name="keep", bufs=1))

        ones_col = keep.tile([P, 1], bf16)
        nc.gpsimd.memset(ones_col, 1.0)
        mhalf_row = keep.tile([1, IB], bf16)
        nc.gpsimd.memset(mhalf_row, -0.5)
        zrow = keep.tile([1, IB], bf16)
        nc.gpsimd.memset(zrow, 0.0)

        # k-major coefficient tensors; dim1 = k half (k, k+128)
        Wb = keep.tile([P, 2, J], bf16)
        G2b = keep.tile([P, 2, J], bf16)
        Qb = keep.tile([P, 2, J], bf16)
        ATb = keep.tile([P, 2, IB], bf16)
        BTb = keep.tile([P, 2, IB], bf16)

        # stationary (block-diag) + moving tiles, two partition row-groups
        # (base 0 / 32) so LDWEIGHTS overlaps in-flight matmuls, chunked so
        # the loop can start before all gathers land
        LHS_E = [keep.tile([6, CHB * P], bf16, tag=f"lhse{q}", name=f"lhse{q}") for q in range(NCH)]
        RHS_E = [keep.tile([6, CHB * J], bf16, tag=f"rhse{q}", name=f"rhse{q}") for q in range(NCH)]
        LHS_Of = [keep.tile([38, CHB * P], bf16, tag=f"lhso{q}", name=f"lhso{q}") for q in range(NCH)]
        RHS_Of = [keep.tile([38, CHB * J], bf16, tag=f"rhso{q}", name=f"rhso{q}") for q in range(NCH)]

        A_red = keep.tile([P, KK], f32)
        ACCR = keep.tile([P, NRC * 3], f32)
        qvS = keep.tile([1, J], bf16)
        OS1 = keep.tile([IB, 2], f32)
        negmax = keep.tile([IB, 1], f32)

        LHSvE = [t.rearrange("r (g n) -> r g n", g=CHB) for t in LHS_E]
        RHSvE = [t.rearrange("r (g n) -> r g n", g=CHB) for t in RHS_E]
        LHSvO = [t[32:38].rearrange("r (g n) -> r g n", g=CHB) for t in LHS_Of]
        RHSvO = [t[32:38].rearrange("r (g n) -> r g n", g=CHB) for t in RHS_Of]

        # ---------------- prep ----------------
        with tc.tile_pool(name="prep", bufs=1) as prep:
            MT = prep.tile([P, 2, J], f32)
            nc.sync.dma_start(MT, d_muT.rearrange("(t p) j -> p t j", p=P))
            LVT = prep.tile([P, 2, J], f32)
            nc.sync.dma_start(LVT, d_lvT.rearrange("(t p) j -> p t j", p=P))
            ST = prep.tile([P, 2, IB], f32)
            nc.sync.dma_start(ST, d_latT.rearrange("(t p) i -> p t i", p=P))
            MTf = MT.rearrange("p t j -> p (t j)")
            LVf = LVT.rearrange("p t j -> p (t j)")
            STf = ST.rearrange("p t i -> p (t i)")

            # coefficients (all in k-major layout, cast to bf16 on write)
            WS = prep.tile([P, 2 * J], f32)
            nc.scalar.activation(WS, LVf, AF.Exp)
            _vcopy(nc, Wb.rearrange("p t j -> p (t j)"), WS)
            nc.vector.scalar_tensor_tensor(
                G2b.rearrange("p t j -> p (t j)"), MTf, 1.0, WS, OP.mult, OP.mult)
            QF = prep.tile([P, 2 * J], f32)
            nc.vector.scalar_tensor_tensor(
                QF, MTf, 1.0, G2b.rearrange("p t j -> p (t j)"), OP.mult, OP.mult)
            nc.vector.scalar_tensor_tensor(
                Qb.rearrange("p t j -> p (t j)"), QF, LOG2PI, LVf, OP.add, OP.add)

            SSQ = prep.tile([P, 2 * IB], f32)
            nc.vector.scalar_tensor_tensor(SSQ, STf, 1.0, STf, OP.mult, OP.mult)
            nc.vector.tensor_scalar(ATb.rearrange("p t i -> p (t i)"), SSQ,
                                    -0.5, None, OP.mult)
            _vcopy(nc, BTb.rearrange("p t i -> p (t i)"), STf)

            # dimension-wise KL partials (full sums, layout-independent)
            DW = prep.tile([P, 2], f32)
            MSQ = prep.tile([P, 2 * J], f32)
            nc.vector.scalar_tensor_tensor(MSQ, MTf, 1.0, MTf, OP.mult, OP.mult)
            nc.vector.scalar_tensor_tensor(MSQ, MSQ, 0.0, LVf, OP.add, OP.add)
            nc.scalar.activation(MSQ, MSQ, AF.Exp, accum_out=DW[:, 0:1])
            nc.vector.tensor_scalar(MSQ, LVf, 1.0, None, OP.mult, OP.add,
                                    accum_out=DW[:, 1:2])
            nc.sync.dma_start(o_dwkl, DW)

            # gathers, chunked; all on the SP (HWDGE) queue engine
            zbcast = bass.AP(tensor=zrow.tensor, offset=zrow.offset,
                             ap=[list(zrow.ap[0]), [0, CHB], [1, IB]])
            mbcast = bass.AP(tensor=mhalf_row.tensor, offset=mhalf_row.offset,
                             ap=[list(mhalf_row.ap[0]), [0, CHB], [1, IB]])
            gate = None
            for q in range(NCH):
                for half, (RHSq, LHSq) in enumerate(
                        ((RHSvE[q], LHSvE[q]), (RHSvO[q], LHSvO[q]))):
                    psl = slice(half * HK + q * CHB, half * HK + (q + 1) * CHB)
                    for r, (srcb, kt) in enumerate(
                            ((Wb, 0), (G2b, 0), (Qb, 0), (Wb, 1), (G2b, 1), (Qb, 1))):
                        nc.sync.dma_start(RHSq[r:r + 1], srcb[psl, kt, :])
                    nc.sync.dma_start(LHSq[0:1, :, 0:IB], ATb[psl, 0, :])
                    nc.sync.dma_start(LHSq[1:2, :, 0:IB], BTb[psl, 0, :])
                    nc.sync.dma_start(LHSq[2:3, :, 0:IB], mbcast)
                    nc.sync.dma_start(LHSq[3:4, :, IB:P], ATb[psl, 1, :])
                    nc.sync.dma_start(LHSq[4:5, :, IB:P], BTb[psl, 1, :])
                    nc.sync.dma_start(LHSq[5:6, :, IB:P], mbcast)
                    # zero halves
                    nc.sync.dma_start(LHSq[0:1, :, IB:P], zbcast)
                    nc.sync.dma_start(LHSq[1:2, :, IB:P], zbcast)
                    nc.sync.dma_start(LHSq[2:3, :, IB:P], zbcast)
                    nc.sync.dma_start(LHSq[3:4, :, 0:IB], zbcast)
                    nc.sync.dma_start(LHSq[4:5, :, 0:IB], zbcast)
                    gate = nc.sync.dma_start(LHSq[5:6, :, 0:IB], zbcast)

        # ---------------- logqz path (S1 = sum_k logqz_mat) ----------------
        with tc.tile_pool(name="s1psum", bufs=1, space="PSUM") as s1p, \
                tc.tile_pool(name="s1sb", bufs=1) as s1sb:
            qpv = s1p.tile([1, J], f32)
            nc.tensor.matmul(qpv, ones_col, Qb[:, 0, :], start=True, stop=False)
            nc.tensor.matmul(qpv, ones_col, Qb[:, 1, :], start=False, stop=True)
            _vcopy(nc, qvS, qpv)

            S1 = s1p.tile([IB, J], f32)
            nc.tensor.matmul(S1, ATb[:, 0, :], Wb[:, 0, :], start=True, stop=False)
            nc.tensor.matmul(S1, BTb[:, 0, :], G2b[:, 0, :], start=False, stop=False)
            nc.tensor.matmul(S1, ATb[:, 1, :], Wb[:, 1, :], start=False, stop=False)
            nc.tensor.matmul(S1, BTb[:, 1, :], G2b[:, 1, :], start=False, stop=False)
            nc.tensor.matmul(S1, mhalf_row, qvS, start=False, stop=True)

            nc.vector.tensor_reduce(negmax, S1, axis=AX.X, op=OP.max, negate=True)
            es = s1sb.tile([IB, J], bf16)
            nc.scalar.activation(es, S1, AF.Exp, bias=negmax, scale=1.0,
                                 accum_out=OS1[:, 1:2])
            _vcopy(nc, OS1[:, 0:1], negmax)
            nc.sync.dma_start(o_s1, OS1)

        # ---------------- main pairwise loop (rec BCE interleaved) --------
        NGG = KK // 8
        rec_at = {2 + 2 * c: c for c in range(NRC)}  # double-group idx -> chunk
        with tc.tile_pool(name="mpsum", bufs=2, space="PSUM") as mp, \
                tc.tile_pool(name="epool", bufs=2) as ep, \
                tc.tile_pool(name="rpool", bufs=2) as rp, \
                tc.tile_pool(name="rpool1", bufs=1) as rp1:
            for gg in range(NGG):
                E8 = ep.tile([P, 8, J], bf16)
                for sub in range(2):
                    T4 = mp.tile([P, 4, J], f32, tag="t4")
                    for c in range(4):
                        m = 8 * gg + 4 * sub + c
                        h = m // 2
                        q, off = h // CHB, h % CHB
                        if m % 2 == 0:
                            lhs, rhs = LHSvE[q][:, off, :], RHSvE[q][:, off, :]
                        else:
                            lhs, rhs = LHSvO[q][:, off, :], RHSvO[q][:, off, :]
                        nc.tensor.matmul(T4[:, c, :], lhs, rhs,
                                         start=True, stop=True)
                    nc.scalar.activation(
                        E8[:, 4 * sub:4 * sub + 4, :].rearrange(
                            "p c j -> p (c j)"),
                        T4.rearrange("p c j -> p (c j)"), AF.Exp)
                hh = J // 2
                while hh >= 16:
                    nc.vector.tensor_add(E8[:, :, 0:hh], E8[:, :, 0:hh],
                                         E8[:, :, hh:2 * hh])
                    hh //= 2
                nc.vector.tensor_reduce(A_red[:, 8 * gg:8 * gg + 8],
                                        E8[:, :, 0:16], axis=AX.X, op=OP.add)

                if gg in rec_at:
                    ch = rec_at[gg]
                    sl = slice(ch * RCH, (ch + 1) * RCH)
                    DD = rp.tile([P, RCH], f32)
                    dd_i = nc.sync.dma_start(DD, d_data[:, sl])
                    tile.add_dep_helper(dd_i.ins, gate.ins, sync=True,
                                        reason="rec prefetch after gathers")
                    RR = rp.tile([P, RCH], f32)
                    rr_i = nc.sync.dma_start(RR, d_rec[:, sl])
                    tile.add_dep_helper(rr_i.ins, gate.ins, sync=True,
                                        reason="rec prefetch after gathers")
                    DDb = rp1.tile([P, RCH], bf16)
                    _vcopy(nc, DDb, DD)
                    LR = rp1.tile([P, RCH], bf16)
                    nc.scalar.activation(LR, RR, AF.Ln)
                    L1R = rp1.tile([P, RCH], bf16)
                    nc.scalar.activation(L1R, RR, AF.Ln, bias=1.0, scale=-1.0,
                                         accum_out=ACCR[:, 3 * ch + 1:3 * ch + 2])
                    nc.vector.scalar_tensor_tensor(
                        LR, DDb, 1.0, LR, OP.mult, OP.mult,
                        accum_out=ACCR[:, 3 * ch:3 * ch + 1])
                    nc.vector.scalar_tensor_tensor(
                        LR, DDb, -1.0, L1R, OP.mult, OP.mult,
                        accum_out=ACCR[:, 3 * ch + 2:3 * ch + 3])
        nc.sync.dma_start(o_rec, ACCR)

        LG = keep.tile([P, KK], f32)
        nc.scalar.activation(LG, A_red, AF.Ln)
        PM = keep.tile([P, 1], f32)
        nc.vector.reduce_sum(PM, LG, axis=AX.X)
        nc.sync.dma_start(o_pm, PM)

    return _split_multi_waits(nc)


def make_in_maps(data, recon, lat, mu, lv):
    muT = np.ascontiguousarray(np.asarray(mu, np.float32).T)
    lvT = np.ascontiguousarray(np.asarray(lv, np.float32).T)
    latT = np.asarray(lat, np.float32).T
    in_maps = []
    for c in range(NCORES):
        sl = slice(c * IB, (c + 1) * IB)
        in_maps.append({
            "muT": muT,
            "lvT": lvT,
            "latT": np.ascontiguousarray(latT[:, sl]),
            "data": np.ascontiguousarray(
                np.asarray(data[sl], np.float32).reshape(P, REC_F)),
            "recon": np.ascontiguousarray(
                np.asarray(recon[sl], np.float32).reshape(P, REC_F)),
        })
    return in_maps


def combine(results, dataset_size):
    """results: list of 8 dicts with per-core output tensors."""
    log_norm = float(np.log(np.float32(B)) + np.log(np.float32(float(dataset_size))))

    rec_sum = sum(r["o_rec"].astype(np.float64).sum() for r in results)
    rec_loss = -rec_sum / B

    dw = results[0]["o_dwkl"].astype(np.float64)
    dwkl = (0.5 * dw[:, 0].sum() - 0.5 * dw[:, 1].sum() - 0.5 * B * Z) / B

    tc_total = 0.0
    for r in results:
        pmh = r["o_pm"].astype(np.float64).ravel()
        pm = pmh[:IB] + pmh[IB:]
        prodmarg = pm - Z * log_norm
        s1 = r["o_s1"].astype(np.float64)
        lq = (-s1[:, 0]) + np.log(s1[:, 1]) - log_norm
        tc_total += (lq - prodmarg).sum()
    tc_loss = tc_total / B

    return np.array(rec_loss + tc_loss + dwkl, dtype=np.float32)


def run_on_hw(inputs, trace=False):
    from concourse.bass_utils import run_bass_kernel_spmd

    nc = build_program()
    in_maps = make_in_maps(inputs["data"], inputs["recon_batch"],
                           inputs["latent_sample"], inputs["mu"],
                           inputs["logvar"])
    br = run_bass_kernel_spmd(nc, in_maps, list(range(NCORES)), trace=trace)
    elbo = combine(br.results, inputs["dataset_size"])
    return elbo, br


def kernel(**inputs):
    elbo, _ = run_on_hw(inputs, trace=False)
    return elbo


# revision 19
# speedup vs baseline: 1.5971x; 1.1714x over previous
"""Trainium2 Bass kernel for nn_BatchTCLoss (beta-TCVAE ELBO loss).

Strategy (8 NeuronCores, data-parallel over the sample axis i):
  - Each core owns 64 of the 512 latent rows (and the matching 64 images for
    the BCE term); mu/logvar are replicated.
  - logqz_mat[i,j,k] = -0.5*((s_ik-mu_jk)^2*exp(lv_jk) + lv_jk + LOG2PI)
    expands as a_ik*w_jk + b_ik*g2_jk + c*q_jk with
      a = -0.5*s^2, b = s, c = -0.5
      w = exp(lv), g2 = mu*w, q = mu^2*w + lv + LOG2PI
    so each (i, k)-slice over all j is a rank-3 matmul.  Two k-slices are
    packed per 128x512 PSUM tile via a 6-row block-diagonal lhsT, giving
    full-width TensorE + ScalarE tiles.
  - Per-(i,k) logsumexp over j: exp on ScalarE (values are <= exp(-0.69), no
    max-subtraction needed), row-sum fused into VectorE tensor_scalar
    accumulators, log at the end.
  - logqz: S1[i,j] = sum_k logqz_mat via 5 accumulated matmuls, then a
    max-stabilized exp-sum on one 64x512 tile.
  - BCE + dimension-wise KL are streamed elementwise reductions.
  - Each core emits tiny per-core partial tensors; the host combines them
    (the final reduction is O(1000) flops).
"""

import numpy as np
from contextlib import ExitStack

import concourse.bass as bass
import concourse.tile as tile
from concourse import mybir
from concourse.masks import make_identity

B = 512          # batch
Z = 256          # latent dim
NCORES = 8
IB = B // NCORES   # 64 local samples per core
J = B              # pairwise j axis
P = 128            # partitions
KK = Z // 2        # 128 k-pairs (k, k+128)
CHW = 3 * 64 * 64
REC_F = IB * CHW // P   # 6144 free elems/partition of the image shard
RCH = 1024              # rec chunk (free elems per partition)
NRC = REC_F // RCH      # 6 chunks
LOG2PI = float(np.log(2.0 * np.pi))

f32 = mybir.dt.float32
bf16 = mybir.dt.bfloat16
AF = mybir.ActivationFunctionType
OP = mybir.AluOpType
AX = mybir.AxisListType




def _vmul(nc, out, a, b):
    # a*b via scalar_tensor_tensor: (a mult 1.0) mult b  (TT encoding has
    # only one sync-wait slot in walrus; TensorScalarPtr has more)
    nc.vector.scalar_tensor_tensor(out, a, 1.0, b, OP.mult, OP.mult)


def _vadd(nc, out, a, b):
    nc.vector.scalar_tensor_tensor(out, a, 0.0, b, OP.add, OP.add)


def _vcopy(nc, out, in_):
    nc.vector.tensor_scalar(out, in_, 0.0, None, OP.add)


def _split_multi_waits(nc):
    """This container's walrus accepts only ONE embedded sync-wait per
    compute/DMA instruction ("Too many sync wait commands").  Hoist extra
    waits onto same-engine NoOp carriers inserted immediately before the
    instruction — engines execute their stream in order, so this is
    semantics-preserving."""
    wid = 0
    for f in nc.m.functions:
        for blk in f.blocks:
            il = blk.instructions
            i = 0
            while i < len(il):
                ins = il[i]
                si = ins.sync_info
                tname = type(ins).__name__
                if si is not None and len(si.on_wait) > 1 and tname != "InstNoOp":
                    waits = list(si.on_wait)
                    nops = []
                    for w in waits[:-1]:
                        nop = mybir.InstNoOp(name=f"WSPLIT-{wid}", ins=[],
                                             outs=[], text_hint="wait_split")
                        wid += 1
                        nop.engine = ins.engine
                        nop.sync_info = mybir.SyncInfo(on_wait=[w], on_update=[])
                        nc.register_instruction(nop, overwrite=True)
                        nops.append(nop)
                    ins.sync_info = mybir.SyncInfo(on_wait=[waits[-1]],
                                                   on_update=list(si.on_update))
                    for j, nop in enumerate(nops):
                        il.insert(i + j, nop)
                    i += len(nops)
                i += 1
    return nc


def build_program():
    nc = bass.Bass("TRN2", target_bir_lowering=False, debug=False)

    # host supplies k-major (transposed) copies of mu/logvar/latent —
    # pure layout work, part of sharding
    d_muT = nc.dram_tensor("muT", [Z, B], f32, kind="ExternalInput").ap()
    d_lvT = nc.dram_tensor("lvT", [Z, B], f32, kind="ExternalInput").ap()
    d_latT = nc.dram_tensor("latT", [Z, IB], f32, kind="ExternalInput").ap()
    d_data = nc.dram_tensor("data", [P, REC_F], f32, kind="ExternalInput").ap()
    d_rec = nc.dram_tensor("recon", [P, REC_F], f32, kind="ExternalInput").ap()

    o_pm = nc.dram_tensor("o_pm", [P, 1], f32, kind="ExternalOutput").ap()
    o_s1 = nc.dram_tensor("o_s1", [IB, 2], f32, kind="ExternalOutput").ap()
    o_rec = nc.dram_tensor("o_rec", [P, NRC * 3], f32, kind="ExternalOutput").ap()
    o_dwkl = nc.dram_tensor("o_dwkl", [P, 2], f32, kind="ExternalOutput").ap()

    HK = KK // 2   # 64 process indices per row-group half
    NCH = 4        # gather chunks per half
    CHB = HK // NCH  # 16 kk-blocks per chunk

    with tile.TileContext(nc) as tc, ExitStack() as ctx:
        keep = ctx.enter_context(tc.tile_pool(name="keep", bufs=1))

        ones_col = keep.tile([P, 1], bf16)
        nc.gpsimd.memset(ones_col, 1.0)
        mhalf_row = keep.tile([1, IB], bf16)
        nc.gpsimd.memset(mhalf_row, -0.5)

        # k-major coefficient tensors; dim1 = k half (k, k+128)
        Wb = keep.tile([P, 2, J], bf16)
        G2b = keep.tile([P, 2, J], bf16)
        Qb = keep.tile([P, 2, J], bf16)
        ATb = keep.tile([P, 2, IB], bf16)
        BTb = keep.tile([P, 2, IB], bf16)

        # stationary (block-diag) + moving tiles, two partition row-groups
        # (base 0 / 32) so LDWEIGHTS overlaps in-flight matmuls, chunked so
        # the loop can start before all gathers land
        LHS_E = [keep.tile([6, CHB * P], bf16, tag=f"lhse{q}", name=f"lhse{q}") for q in range(NCH)]
        RHS_E = [keep.tile([6, CHB * J], bf16, tag=f"rhse{q}", name=f"rhse{q}") for q in range(NCH)]
        LHS_Of = [keep.tile([38, CHB * P], bf16, tag=f"lhso{q}", name=f"lhso{q}") for q in range(NCH)]
        RHS_Of = [keep.tile([38, CHB * J], bf16, tag=f"rhso{q}", name=f"rhso{q}") for q in range(NCH)]

        A_red = keep.tile([P, KK], f32)
        ACCR = keep.tile([P, NRC * 3], f32)
        qvS = keep.tile([1, J], bf16)
        OS1 = keep.tile([IB, 2], f32)
        negmax = keep.tile([IB, 1], f32)

        LHSvE = [t.rearrange("r (g n) -> r g n", g=CHB) for t in LHS_E]
        RHSvE = [t.rearrange("r (g n) -> r g n", g=CHB) for t in RHS_E]
        LHSvO = [t[32:38].rearrange("r (g n) -> r g n", g=CHB) for t in LHS_Of]
        RHSvO = [t[32:38].rearrange("r (g n) -> r g n", g=CHB) for t in RHS_Of]

        # ---------------- prep ----------------
        with tc.tile_pool(name="prep", bufs=1) as prep:
            MT = prep.tile([P, 2, J], f32)
            nc.sync.dma_start(MT, d_muT.rearrange("(t p) j -> p t j", p=P))
            LVT = prep.tile([P, 2, J], f32)
            nc.sync.dma_start(LVT, d_lvT.rearrange("(t p) j -> p t j", p=P))
            ST = prep.tile([P, 2, IB], f32)
            nc.sync.dma_start(ST, d_latT.rearrange("(t p) i -> p t i", p=P))
            MTf = MT.rearrange("p t j -> p (t j)")
            LVf = LVT.rearrange("p t j -> p (t j)")
            STf = ST.rearrange("p t i -> p (t i)")

            # coefficients (all in k-major layout, cast to bf16 on write)
            WS = prep.tile([P, 2 * J], f32)
            nc.scalar.activation(WS, LVf, AF.Exp)
            _vcopy(nc, Wb.rearrange("p t j -> p (t j)"), WS)
            nc.vector.scalar_tensor_tensor(
                G2b.rearrange("p t j -> p (t j)"), MTf, 1.0, WS, OP.mult, OP.mult)
            QF = prep.tile([P, 2 * J], f32)
            nc.vector.scalar_tensor_tensor(
                QF, MTf, 1.0, G2b.rearrange("p t j -> p (t j)"), OP.mult, OP.mult)
            nc.vector.scalar_tensor_tensor(
                Qb.rearrange("p t j -> p (t j)"), QF, LOG2PI, LVf, OP.add, OP.add)

            SSQ = prep.tile([P, 2 * IB], f32)
            nc.vector.scalar_tensor_tensor(SSQ, STf, 1.0, STf, OP.mult, OP.mult)
            nc.vector.tensor_scalar(ATb.rearrange("p t i -> p (t i)"), SSQ,
                                    -0.5, None, OP.mult)
            _vcopy(nc, BTb.rearrange("p t i -> p (t i)"), STf)

            # dimension-wise KL partials (full sums, layout-independent)
            DW = prep.tile([P, 2], f32)
            MSQ = prep.tile([P, 2 * J], f32)
            nc.vector.scalar_tensor_tensor(MSQ, MTf, 1.0, MTf, OP.mult, OP.mult)
            nc.vector.scalar_tensor_tensor(MSQ, MSQ, 0.0, LVf, OP.add, OP.add)
            nc.scalar.activation(MSQ, MSQ, AF.Exp, accum_out=DW[:, 0:1])
            nc.vector.tensor_scalar(MSQ, LVf, 1.0, None, OP.mult, OP.add,
                                    accum_out=DW[:, 1:2])
            nc.sync.dma_start(o_dwkl, DW)

            # gathers, chunked; alternate between the two DMA-issue engines
            mbcast = bass.AP(tensor=mhalf_row.tensor, offset=mhalf_row.offset,
                             ap=[list(mhalf_row.ap[0]), [0, CHB], [1, IB]])
            dq = [nc.sync, nc.gpsimd]
            qi = 0
            # zero-fill whole stationary tiles first (their base partitions
            # are 0/32, so a plain engine memset is legal); gathers overwrite
            # the data regions afterwards
            for q in range(NCH):
                nc.vector.memset(LHS_E[q], 0.0)
                nc.vector.memset(LHS_Of[q][32:38], 0.0)
            for q in range(NCH):
                for half, (RHSq, LHSq) in enumerate(
                        ((RHSvE[q], LHSvE[q]), (RHSvO[q], LHSvO[q]))):
                    psl = slice(half * HK + q * CHB, half * HK + (q + 1) * CHB)
                    for r, (srcb, kt) in enumerate(
                            ((Wb, 0), (G2b, 0), (Qb, 0), (Wb, 1), (G2b, 1), (Qb, 1))):
                        dq[qi % 2].dma_start(RHSq[r:r + 1], srcb[psl, kt, :])
                        qi += 1
                    dq[qi % 2].dma_start(LHSq[0:1, :, 0:IB], ATb[psl, 0, :]); qi += 1
                    dq[qi % 2].dma_start(LHSq[1:2, :, 0:IB], BTb[psl, 0, :]); qi += 1
                    dq[qi % 2].dma_start(LHSq[2:3, :, 0:IB], mbcast); qi += 1
                    dq[qi % 2].dma_start(LHSq[3:4, :, IB:P], ATb[psl, 1, :]); qi += 1
                    dq[qi % 2].dma_start(LHSq[4:5, :, IB:P], BTb[psl, 1, :]); qi += 1
                    dq[qi % 2].dma_start(LHSq[5:6, :, IB:P], mbcast); qi += 1

        # ---------------- logqz path (S1 = sum_k logqz_mat) ----------------
        with tc.tile_pool(name="s1psum", bufs=1, space="PSUM") as s1p, \
                tc.tile_pool(name="s1sb", bufs=1) as s1sb:
            qpv = s1p.tile([1, J], f32)
            nc.tensor.matmul(qpv, ones_col, Qb[:, 0, :], start=True, stop=False)
            nc.tensor.matmul(qpv, ones_col, Qb[:, 1, :], start=False, stop=True)
            _vcopy(nc, qvS, qpv)

            S1 = s1p.tile([IB, J], f32)
            nc.tensor.matmul(S1, ATb[:, 0, :], Wb[:, 0, :], start=True, stop=False)
            nc.tensor.matmul(S1, BTb[:, 0, :], G2b[:, 0, :], start=False, stop=False)
            nc.tensor.matmul(S1, ATb[:, 1, :], Wb[:, 1, :], start=False, stop=False)
            nc.tensor.matmul(S1, BTb[:, 1, :], G2b[:, 1, :], start=False, stop=False)
            nc.tensor.matmul(S1, mhalf_row, qvS, start=False, stop=True)

            nc.vector.tensor_reduce(negmax, S1, axis=AX.X, op=OP.max, negate=True)
            es = s1sb.tile([IB, J], bf16)
            nc.scalar.activation(es, S1, AF.Exp, bias=negmax, scale=1.0,
                                 accum_out=OS1[:, 1:2])
            _vcopy(nc, OS1[:, 0:1], negmax)
            nc.sync.dma_start(o_s1, OS1)

        # ---------------- main pairwise loop (rec BCE interleaved) --------
        NGG = KK // 8
        rec_at = {2 + 2 * c: c for c in range(NRC)}  # double-group idx -> chunk
        with tc.tile_pool(name="mpsum", bufs=2, space="PSUM") as mp, \
                tc.tile_pool(name="epool", bufs=2) as ep, \
                tc.tile_pool(name="rpool", bufs=2) as rp, \
                tc.tile_pool(name="rpool1", bufs=1) as rp1:
            for gg in range(NGG):
                E8 = ep.tile([P, 8, J], bf16)
                for sub in range(2):
                    T4 = mp.tile([P, 4, J], f32, tag="t4")
                    for c in range(4):
                        m = 8 * gg + 4 * sub + c
                        h = m // 2
                        q, off = h // CHB, h % CHB
                        if m % 2 == 0:
                            lhs, rhs = LHSvE[q][:, off, :], RHSvE[q][:, off, :]
                        else:
                            lhs, rhs = LHSvO[q][:, off, :], RHSvO[q][:, off, :]
                        nc.tensor.matmul(T4[:, c, :], lhs, rhs,
                                         start=True, stop=True)
                    nc.scalar.activation(
                        E8[:, 4 * sub:4 * sub + 4, :].rearrange(
                            "p c j -> p (c j)"),
                        T4.rearrange("p c j -> p (c j)"), AF.Exp)
                hh = J // 2
                while hh >= 16:
                    nc.vector.tensor_add(E8[:, :, 0:hh], E8[:, :, 0:hh],
                                         E8[:, :, hh:2 * hh])
                    hh //= 2
                nc.vector.tensor_reduce(A_red[:, 8 * gg:8 * gg + 8],
                                        E8[:, :, 0:16], axis=AX.X, op=OP.add)

                if gg in rec_at:
                    ch = rec_at[gg]
                    sl = slice(ch * RCH, (ch + 1) * RCH)
                    DD = rp.tile([P, RCH], f32)
                    nc.gpsimd.dma_start(DD, d_data[:, sl])
                    RR = rp.tile([P, RCH], f32)
                    nc.gpsimd.dma_start(RR, d_rec[:, sl])
                    DDb = rp1.tile([P, RCH], bf16)
                    _vcopy(nc, DDb, DD)
                    LR = rp1.tile([P, RCH], bf16)
                    nc.scalar.activation(LR, RR, AF.Ln)
                    L1R = rp1.tile([P, RCH], bf16)
                    nc.scalar.activation(L1R, RR, AF.Ln, bias=1.0, scale=-1.0,
                                         accum_out=ACCR[:, 3 * ch + 1:3 * ch + 2])
                    nc.vector.scalar_tensor_tensor(
                        LR, DDb, 1.0, LR, OP.mult, OP.mult,
                        accum_out=ACCR[:, 3 * ch:3 * ch + 1])
                    nc.vector.scalar_tensor_tensor(
                        LR, DDb, -1.0, L1R, OP.mult, OP.mult,
                        accum_out=ACCR[:, 3 * ch + 2:3 * ch + 3])
        nc.sync.dma_start(o_rec, ACCR)

        LG = keep.tile([P, KK], f32)
        nc.scalar.activation(LG, A_red, AF.Ln)
        PM = keep.tile([P, 1], f32)
        nc.vector.reduce_sum(PM, LG, axis=AX.X)
        nc.sync.dma_start(o_pm, PM)

    return _split_multi_waits(nc)


def make_in_maps(data, recon, lat, mu, lv):
    muT = np.ascontiguousarray(np.asarray(mu, np.float32).T)
    lvT = np.ascontiguousarray(np.asarray(lv, np.float32).T)
    latT = np.asarray(lat, np.float32).T
    in_maps = []
    for c in range(NCORES):
        sl = slice(c * IB, (c + 1) * IB)
        in_maps.append({
            "muT": muT,
            "lvT": lvT,
            "latT": np.ascontiguousarray(latT[:, sl]),
            "data": np.ascontiguousarray(
                np.asarray(data[sl], np.float32).reshape(P, REC_F)),
            "recon": np.ascontiguousarray(
                np.asarray(recon[sl], np.float32).reshape(P, REC_F)),
        })
    return in_maps


def combine(results, dataset_size):
    """results: list of 8 dicts with per-core output tensors."""
    log_norm = float(np.log(np.float32(B)) + np.log(np.float32(float(dataset_size))))

    rec_sum = sum(r["o_rec"].astype(np.float64).sum() for r in results)
    rec_loss = -rec_sum / B

    dw = results[0]["o_dwkl"].astype(np.float64)
    dwkl = (0.5 * dw[:, 0].sum() - 0.5 * dw[:, 1].sum() - 0.5 * B * Z) / B

    tc_total = 0.0
    for r in results:
        pmh = r["o_pm"].astype(np.float64).ravel()
        pm = pmh[:IB] + pmh[IB:]
        prodmarg = pm - Z * log_norm
        s1 = r["o_s1"].astype(np.float64)
        lq = (-s1[:, 0]) + np.log(s1[:, 1]) - log_norm
        tc_total += (lq - prodmarg).sum()
    tc_loss = tc_total / B

    return np.array(rec_loss + tc_loss + dwkl, dtype=np.float32)


def run_on_hw(inputs, trace=False):
    from concourse.bass_utils import run_bass_kernel_spmd

    nc = build_program()
    in_maps = make_in_maps(inputs["data"], inputs["recon_batch"],
                           inputs["latent_sample"], inputs["mu"],
                           inputs["logvar"])
    br = run_bass_kernel_spmd(nc, in_maps, list(range(NCORES)), trace=trace)
    elbo = combine(br.results, inputs["dataset_size"])
    return elbo, br


def kernel(**inputs):
    elbo, _ = run_on_hw(inputs, trace=False)
    return elbo


# revision 20
# speedup vs baseline: 1.6812x; 1.0526x over previous
"""Trainium2 Bass kernel for nn_BatchTCLoss (beta-TCVAE ELBO loss).

Strategy (8 NeuronCores, data-parallel over the sample axis i):
  - Each core owns 64 of the 512 latent rows (and the matching 64 images for
    the BCE term); mu/logvar are replicated.
  - logqz_mat[i,j,k] = -0.5*((s_ik-mu_jk)^2*exp(lv_jk) + lv_jk + LOG2PI)
    expands as a_ik*w_jk + b_ik*g2_jk + c*q_jk with
      a = -0.5*s^2, b = s, c = -0.5
      w = exp(lv), g2 = mu*w, q = mu^2*w + lv + LOG2PI
    so each (i, k)-slice over all j is a rank-3 matmul.  Two k-slices are
    packed per 128x512 PSUM tile via a 6-row block-diagonal lhsT, giving
    full-width TensorE + ScalarE tiles.
  - Per-(i,k) logsumexp over j: exp on ScalarE (values are <= exp(-0.69), no
    max-subtraction needed), row-sum fused into VectorE tensor_scalar
    accumulators, log at the end.
  - logqz: S1[i,j] = sum_k logqz_mat via 5 accumulated matmuls, then a
    max-stabilized exp-sum on one 64x512 tile.
  - BCE + dimension-wise KL are streamed elementwise reductions.
  - Each core emits tiny per-core partial tensors; the host combines them
    (the final reduction is O(1000) flops).
"""

import numpy as np
from contextlib import ExitStack

import concourse.bass as bass
import concourse.tile as tile
from concourse import mybir
from concourse.masks import make_identity

B = 512          # batch
Z = 256          # latent dim
NCORES = 8
IB = B // NCORES   # 64 local samples per core
J = B              # pairwise j axis
P = 128            # partitions
KK = Z // 2        # 128 k-pairs (k, k+128)
CHW = 3 * 64 * 64
REC_F = IB * CHW // P   # 6144 free elems/partition of the image shard
RCH = 1024              # rec chunk (free elems per partition)
NRC = REC_F // RCH      # 6 chunks
LOG2PI = float(np.log(2.0 * np.pi))

f32 = mybir.dt.float32
bf16 = mybir.dt.bfloat16
AF = mybir.ActivationFunctionType
OP = mybir.AluOpType
AX = mybir.AxisListType




def _vmul(nc, out, a, b):
    # a*b via scalar_tensor_tensor: (a mult 1.0) mult b  (TT encoding has
    # only one sync-wait slot in walrus; TensorScalarPtr has more)
    nc.vector.scalar_tensor_tensor(out, a, 1.0, b, OP.mult, OP.mult)


def _vadd(nc, out, a, b):
    nc.vector.scalar_tensor_tensor(out, a, 0.0, b, OP.add, OP.add)


def _vcopy(nc, out, in_):
    nc.vector.tensor_scalar(out, in_, 0.0, None, OP.add)


def _split_multi_waits(nc):
    """This container's walrus accepts only ONE embedded sync-wait per
    compute/DMA instruction ("Too many sync wait commands").  Hoist extra
    waits onto same-engine NoOp carriers inserted immediately before the
    instruction — engines execute their stream in order, so this is
    semantics-preserving."""
    wid = 0
    for f in nc.m.functions:
        for blk in f.blocks:
            il = blk.instructions
            i = 0
            while i < len(il):
                ins = il[i]
                si = ins.sync_info
                tname = type(ins).__name__
                if si is not None and len(si.on_wait) > 1 and tname != "InstNoOp":
                    waits = list(si.on_wait)
                    nops = []
                    for w in waits[:-1]:
                        nop = mybir.InstNoOp(name=f"WSPLIT-{wid}", ins=[],
                                             outs=[], text_hint="wait_split")
                        wid += 1
                        nop.engine = ins.engine
                        nop.sync_info = mybir.SyncInfo(on_wait=[w], on_update=[])
                        nc.register_instruction(nop, overwrite=True)
                        nops.append(nop)
                    ins.sync_info = mybir.SyncInfo(on_wait=[waits[-1]],
                                                   on_update=list(si.on_update))
                    for j, nop in enumerate(nops):
                        il.insert(i + j, nop)
                    i += len(nops)
                i += 1
    return nc


def build_program():
    nc = bass.Bass("TRN2", target_bir_lowering=False, debug=False)

    # host supplies k-major (transposed) copies of mu/logvar/latent —
    # pure layout work, part of sharding
    d_muT = nc.dram_tensor("muT", [Z, B], f32, kind="ExternalInput").ap()
    d_lvT = nc.dram_tensor("lvT", [Z, B], f32, kind="ExternalInput").ap()
    d_latT = nc.dram_tensor("latT", [Z, IB], f32, kind="ExternalInput").ap()
    d_data = nc.dram_tensor("data", [P, REC_F], f32, kind="ExternalInput").ap()
    d_rec = nc.dram_tensor("recon", [P, REC_F], f32, kind="ExternalInput").ap()

    o_pm = nc.dram_tensor("o_pm", [P, 1], f32, kind="ExternalOutput").ap()
    o_s1 = nc.dram_tensor("o_s1", [IB, 2], f32, kind="ExternalOutput").ap()
    o_rec = nc.dram_tensor("o_rec", [P, NRC * 3], f32, kind="ExternalOutput").ap()
    o_dwkl = nc.dram_tensor("o_dwkl", [P, 2], f32, kind="ExternalOutput").ap()

    HK = KK // 2   # 64 process indices per row-group half
    NCH = 4        # gather chunks per half
    CHB = HK // NCH  # 16 kk-blocks per chunk

    with tile.TileContext(nc) as tc, ExitStack() as ctx:
        keep = ctx.enter_context(tc.tile_pool(name="keep", bufs=1))

        ones_col = keep.tile([P, 1], bf16)
        nc.gpsimd.memset(ones_col, 1.0)
        mhalf_row = keep.tile([1, IB], bf16)
        nc.gpsimd.memset(mhalf_row, -0.5)

        # k-major coefficient tensors; dim1 = k half (k, k+128)
        Wb = keep.tile([P, 2, J], bf16)
        G2b = keep.tile([P, 2, J], bf16)
        Qb = keep.tile([P, 2, J], bf16)
        ATb = keep.tile([P, 2, IB], bf16)
        BTb = keep.tile([P, 2, IB], bf16)

        # stationary (block-diag) + moving tiles, two partition row-groups
        # (base 0 / 32) so LDWEIGHTS overlaps in-flight matmuls, chunked so
        # the loop can start before all gathers land
        LHS_E = [keep.tile([6, CHB * P], bf16, tag=f"lhse{q}", name=f"lhse{q}") for q in range(NCH)]
        RHS_E = [keep.tile([6, CHB * J], bf16, tag=f"rhse{q}", name=f"rhse{q}") for q in range(NCH)]
        LHS_Of = [keep.tile([38, CHB * P], bf16, tag=f"lhso{q}", name=f"lhso{q}") for q in range(NCH)]
        RHS_Of = [keep.tile([38, CHB * J], bf16, tag=f"rhso{q}", name=f"rhso{q}") for q in range(NCH)]

        A_red = keep.tile([P, KK], f32)
        LG = keep.tile([P, KK], f32)
        PMH = keep.tile([P, 2], f32)
        ACCR = keep.tile([P, NRC * 3], f32)
        qvS = keep.tile([1, J], bf16)
        OS1 = keep.tile([IB, 2], f32)
        negmax = keep.tile([IB, 1], f32)

        LHSvE = [t.rearrange("r (g n) -> r g n", g=CHB) for t in LHS_E]
        RHSvE = [t.rearrange("r (g n) -> r g n", g=CHB) for t in RHS_E]
        LHSvO = [t[32:38].rearrange("r (g n) -> r g n", g=CHB) for t in LHS_Of]
        RHSvO = [t[32:38].rearrange("r (g n) -> r g n", g=CHB) for t in RHS_Of]

        # ---------------- prep ----------------
        with tc.tile_pool(name="prep", bufs=1) as prep:
            MT = prep.tile([P, 2, J], f32)
            nc.sync.dma_start(MT, d_muT.rearrange("(t p) j -> p t j", p=P))
            LVT = prep.tile([P, 2, J], f32)
            nc.sync.dma_start(LVT, d_lvT.rearrange("(t p) j -> p t j", p=P))
            ST = prep.tile([P, 2, IB], f32)
            nc.sync.dma_start(ST, d_latT.rearrange("(t p) i -> p t i", p=P))
            MTf = MT.rearrange("p t j -> p (t j)")
            LVf = LVT.rearrange("p t j -> p (t j)")
            STf = ST.rearrange("p t i -> p (t i)")

            # coefficients (all in k-major layout, cast to bf16 on write)
            WS = prep.tile([P, 2 * J], f32)
            nc.scalar.activation(WS, LVf, AF.Exp)
            _vcopy(nc, Wb.rearrange("p t j -> p (t j)"), WS)
            nc.vector.scalar_tensor_tensor(
                G2b.rearrange("p t j -> p (t j)"), MTf, 1.0, WS, OP.mult, OP.mult)
            QF = prep.tile([P, 2 * J], f32)
            nc.vector.scalar_tensor_tensor(
                QF, MTf, 1.0, G2b.rearrange("p t j -> p (t j)"), OP.mult, OP.mult)
            nc.vector.scalar_tensor_tensor(
                Qb.rearrange("p t j -> p (t j)"), QF, LOG2PI, LVf, OP.add, OP.add)

            SSQ = prep.tile([P, 2 * IB], f32)
            nc.vector.scalar_tensor_tensor(SSQ, STf, 1.0, STf, OP.mult, OP.mult)
            nc.vector.tensor_scalar(ATb.rearrange("p t i -> p (t i)"), SSQ,
                                    -0.5, None, OP.mult)
            _vcopy(nc, BTb.rearrange("p t i -> p (t i)"), STf)

            # dimension-wise KL partials (full sums, layout-independent)
            DW = prep.tile([P, 2], f32)
            MSQ = prep.tile([P, 2 * J], f32)
            nc.vector.scalar_tensor_tensor(MSQ, MTf, 1.0, MTf, OP.mult, OP.mult)
            nc.vector.scalar_tensor_tensor(MSQ, MSQ, 0.0, LVf, OP.add, OP.add)
            nc.scalar.activation(MSQ, MSQ, AF.Exp, accum_out=DW[:, 0:1])
            nc.vector.tensor_scalar(MSQ, LVf, 1.0, None, OP.mult, OP.add,
                                    accum_out=DW[:, 1:2])
            nc.sync.dma_start(o_dwkl, DW)

            # gathers, chunked; alternate between the two DMA-issue engines
            mbcast = bass.AP(tensor=mhalf_row.tensor, offset=mhalf_row.offset,
                             ap=[list(mhalf_row.ap[0]), [0, CHB], [1, IB]])
            dq = [nc.sync, nc.gpsimd]
            qi = 0
            # zero-fill whole stationary tiles first (their base partitions
            # are 0/32, so a plain engine memset is legal); gathers overwrite
            # the data regions afterwards
            for q in range(NCH):
                nc.gpsimd.memset(LHS_E[q], 0.0)
                nc.gpsimd.memset(LHS_Of[q][32:38], 0.0)
            for q in range(NCH):
                for half, (RHSq, LHSq) in enumerate(
                        ((RHSvE[q], LHSvE[q]), (RHSvO[q], LHSvO[q]))):
                    psl = slice(half * HK + q * CHB, half * HK + (q + 1) * CHB)
                    for r, (srcb, kt) in enumerate(
                            ((Wb, 0), (G2b, 0), (Qb, 0), (Wb, 1), (G2b, 1), (Qb, 1))):
                        dq[qi % 2].dma_start(RHSq[r:r + 1], srcb[psl, kt, :])
                        qi += 1
                    dq[qi % 2].dma_start(LHSq[0:1, :, 0:IB], ATb[psl, 0, :]); qi += 1
                    dq[qi % 2].dma_start(LHSq[1:2, :, 0:IB], BTb[psl, 0, :]); qi += 1
                    dq[qi % 2].dma_start(LHSq[2:3, :, 0:IB], mbcast); qi += 1
                    dq[qi % 2].dma_start(LHSq[3:4, :, IB:P], ATb[psl, 1, :]); qi += 1
                    dq[qi % 2].dma_start(LHSq[4:5, :, IB:P], BTb[psl, 1, :]); qi += 1
                    dq[qi % 2].dma_start(LHSq[5:6, :, IB:P], mbcast); qi += 1

        # ---------------- logqz path (S1 = sum_k logqz_mat) ----------------
        with tc.tile_pool(name="s1psum", bufs=1, space="PSUM") as s1p, \
                tc.tile_pool(name="s1sb", bufs=1) as s1sb:
            qpv = s1p.tile([1, J], f32)
            nc.tensor.matmul(qpv, ones_col, Qb[:, 0, :], start=True, stop=False)
            nc.tensor.matmul(qpv, ones_col, Qb[:, 1, :], start=False, stop=True)
            _vcopy(nc, qvS, qpv)

            S1 = s1p.tile([IB, J], f32)
            nc.tensor.matmul(S1, ATb[:, 0, :], Wb[:, 0, :], start=True, stop=False)
            nc.tensor.matmul(S1, BTb[:, 0, :], G2b[:, 0, :], start=False, stop=False)
            nc.tensor.matmul(S1, ATb[:, 1, :], Wb[:, 1, :], start=False, stop=False)
            nc.tensor.matmul(S1, BTb[:, 1, :], G2b[:, 1, :], start=False, stop=False)
            nc.tensor.matmul(S1, mhalf_row, qvS, start=False, stop=True)

            nc.vector.tensor_reduce(negmax, S1, axis=AX.X, op=OP.max, negate=True)
            es = s1sb.tile([IB, J], bf16)
            nc.scalar.activation(es, S1, AF.Exp, bias=negmax, scale=1.0,
                                 accum_out=OS1[:, 1:2])
            _vcopy(nc, OS1[:, 0:1], negmax)
            nc.sync.dma_start(o_s1, OS1)

        # ---------------- main pairwise loop (rec BCE interleaved) --------
        NGG = KK // 8
        rec_at = {2 + 2 * c: c for c in range(NRC)}  # double-group idx -> chunk
        with tc.tile_pool(name="mpsum", bufs=2, space="PSUM") as mp, \
                tc.tile_pool(name="epool", bufs=2) as ep, \
                tc.tile_pool(name="rpool", bufs=2) as rp, \
                tc.tile_pool(name="rpool1", bufs=1) as rp1:
            for gg in range(NGG):
                E8 = ep.tile([P, 8, J], bf16)
                for sub in range(2):
                    T4 = mp.tile([P, 4, J], f32, tag="t4")
                    for c in range(4):
                        m = 8 * gg + 4 * sub + c
                        h = m // 2
                        q, off = h // CHB, h % CHB
                        if m % 2 == 0:
                            lhs, rhs = LHSvE[q][:, off, :], RHSvE[q][:, off, :]
                        else:
                            lhs, rhs = LHSvO[q][:, off, :], RHSvO[q][:, off, :]
                        nc.tensor.matmul(T4[:, c, :], lhs, rhs,
                                         start=True, stop=True)
                    nc.scalar.activation(
                        E8[:, 4 * sub:4 * sub + 4, :].rearrange(
                            "p c j -> p (c j)"),
                        T4.rearrange("p c j -> p (c j)"), AF.Exp)
                hh = J // 2
                while hh >= 16:
                    nc.vector.tensor_add(E8[:, :, 0:hh], E8[:, :, 0:hh],
                                         E8[:, :, hh:2 * hh])
                    hh //= 2
                nc.vector.tensor_reduce(A_red[:, 8 * gg:8 * gg + 8],
                                        E8[:, :, 0:16], axis=AX.X, op=OP.add)

                if gg == NGG // 2 - 1:
                    # first half of A_red complete: log+reduce it now so the
                    # post-loop tail only handles the second half
                    nc.scalar.activation(LG[:, 0:KK // 2], A_red[:, 0:KK // 2],
                                         AF.Ln)
                    nc.vector.reduce_sum(PMH[:, 0:1], LG[:, 0:KK // 2],
                                         axis=AX.X)

                if gg in rec_at:
                    ch = rec_at[gg]
                    sl = slice(ch * RCH, (ch + 1) * RCH)
                    DD = rp.tile([P, RCH], f32)
                    nc.gpsimd.dma_start(DD, d_data[:, sl])
                    RR = rp.tile([P, RCH], f32)
                    nc.gpsimd.dma_start(RR, d_rec[:, sl])
                    DDb = rp1.tile([P, RCH], bf16)
                    _vcopy(nc, DDb, DD)
                    LR = rp1.tile([P, RCH], bf16)
                    nc.scalar.activation(LR, RR, AF.Ln)
                    L1R = rp1.tile([P, RCH], bf16)
                    nc.scalar.activation(L1R, RR, AF.Ln, bias=1.0, scale=-1.0,
                                         accum_out=ACCR[:, 3 * ch + 1:3 * ch + 2])
                    nc.vector.scalar_tensor_tensor(
                        LR, DDb, 1.0, LR, OP.mult, OP.mult,
                        accum_out=ACCR[:, 3 * ch:3 * ch + 1])
                    nc.vector.scalar_tensor_tensor(
                        LR, DDb, -1.0, L1R, OP.mult, OP.mult,
                        accum_out=ACCR[:, 3 * ch + 2:3 * ch + 3])
        nc.sync.dma_start(o_rec, ACCR)

        nc.scalar.activation(LG[:, KK // 2:KK], A_red[:, KK // 2:KK], AF.Ln)
        nc.vector.reduce_sum(PMH[:, 1:2], LG[:, KK // 2:KK], axis=AX.X)
        PM = keep.tile([P, 1], f32)
        nc.vector.tensor_scalar(PM, PMH[:, 0:1], 0.0, None, OP.add,
                                accum_out=None)
        nc.vector.scalar_tensor_tensor(PM, PMH[:, 0:1], 0.0, PMH[:, 1:2],
                                       OP.add, OP.add)
        nc.sync.dma_start(o_pm, PM)

    return _split_multi_waits(nc)


def make_in_maps(data, recon, lat, mu, lv):
    muT = np.ascontiguousarray(np.asarray(mu, np.float32).T)
    lvT = np.ascontiguousarray(np.asarray(lv, np.float32).T)
    latT = np.asarray(lat, np.float32).T
    in_maps = []
    for c in range(NCORES):
        sl = slice(c * IB, (c + 1) * IB)
        in_maps.append({
            "muT": muT,
            "lvT": lvT,
            "latT": np.ascontiguousarray(latT[:, sl]),
            "data": np.ascontiguousarray(
                np.asarray(data[sl], np.float32).reshape(P, REC_F)),
            "recon": np.ascontiguousarray(
                np.asarray(recon[sl], np.float32).reshape(P, REC_F)),
        })
    return in_maps


def combine(results, dataset_size):
    """results: list of 8 dicts with per-core output tensors."""
    log_norm = float(np.log(np.float32(B)) + np.log(np.float32(float(dataset_size))))

    rec_sum = sum(r["o_rec"].astype(np.float64).sum() for r in results)
    rec_loss = -rec_sum / B

    dw = results[0]["o_dwkl"].astype(np.float64)
    dwkl = (0.5 * dw[:, 0].sum() - 0.5 * dw[:, 1].sum() - 0.5 * B * Z) / B

    tc_total = 0.0
    for r in results:
        pmh = r["o_pm"].astype(np.float64).ravel()
        pm = pmh[:IB] + pmh[IB:]
        prodmarg = pm - Z * log_norm
        s1 = r["o_s1"].astype(np.float64)
        lq = (-s1[:, 0]) + np.log(s1[:, 1]) - log_norm
        tc_total += (lq - prodmarg).sum()
    tc_loss = tc_total / B

    return np.array(rec_loss + tc_loss + dwkl, dtype=np.float32)


def run_on_hw(inputs, trace=False):
    from concourse.bass_utils import run_bass_kernel_spmd

    nc = build_program()
    in_maps = make_in_maps(inputs["data"], inputs["recon_batch"],
                           inputs["latent_sample"], inputs["mu"],
                           inputs["logvar"])
    br = run_bass_kernel_spmd(nc, in_maps, list(range(NCORES)), trace=trace)
    elbo = combine(br.results, inputs["dataset_size"])
    return elbo, br


def kernel(**inputs):
    elbo, _ = run_on_hw(inputs, trace=False)
    return elbo
